# revision 9
# baseline (speedup 1.0000x reference)
"""Trainium2 Bass kernel for EnhancedLocalAttentionWithGQA (differential
windowed attention, B=2 L=4096 E=1024 H=16 G=2 W=256 D=64).

Structural facts exploited (same as prior version):
  - Only windows 0..15 contribute; core c owns windows (2c, 2c+1) ->
    output rows [512c, 512c+512) per batch; needs x rows [256c, 256c+384).
  - q^T/k^T computed in [head-dim, seq] layout with host-permuted weight
    columns so RoPE is 2 tensor muls + a P32 permutation matmul + add.
  - Branch-2 weights block-swapped so the two differential branches use
    complementary 64-partition halves (K=64 score matmuls).
  - Scores transposed S^T[k, q]; exp without max-subtraction; PV with an
    extra ones column giving the softmax denominators.

This version restructures for engine balance + pipelining:
  - ACT does ONLY the exps (plus cheap at-tile copies); the old
    ACT-copy-with-scale normalize step is replaced by a DVE/GPSIMD
    tensor_scalar + scalar_tensor_tensor pair. The lambda fold is baked
    into the PV rhs: vext = [1/lam | v | 1], so branch-2's denominator
    column comes out pre-divided by lambda and one reciprocal yields
    both branch scales. The combined sign flip (pair = lam*a2' - a1') is
    fixed by negating Wout on the host.
  - RoPE: t-mul + final add on DVE, u-mul on GPSIMD, P32 swap on PE.
  - Software pipelining: per batch, head-group mt's attention stages are
    emitted with a 2-stage lag (scores -> exp -> PV+normalize) and
    q-projections for mt+2 are emitted between stages; batch-1
    projections interleave with batch-0's out-projection so the PE never
    idles (stays at 2.4 GHz).
"""

import os
import sys

sys.path.insert(0, "/opt/trn_rl_repo")
os.environ.setdefault("MYCRO_LOCAL_CACHE", "1")

import numpy as np

B, L, E, H, G, W, D = 2, 4096, 1024, 16, 2, 256, 64
NCORES = 8
SEQ = 384          # x rows per core
NW = 2             # windows per core
QROWS = 512        # output rows per core per batch
KV = E // (H // G)  # 128
LAMBDA_INIT = 0.8


# ----------------------------------------------------------------- host prep

def _head_perm():
    """Column permutation applied to Wq1/Wk1: per 64-block -> [evens|odds]."""
    p = []
    for blk in range(0, E, D):
        p += [blk + 2 * j for j in range(D // 2)]
        p += [blk + 2 * j + 1 for j in range(D // 2)]
    return np.array(p, dtype=np.int64)


def _q2_perm():
    """q2: like _head_perm but heads swapped within each 128-col M-tile."""
    base = _head_perm()
    p = np.empty_like(base)
    for m in range(E // 128):
        p[m * 128: m * 128 + 64] = base[m * 128 + 64: m * 128 + 128]
        p[m * 128 + 64: m * 128 + 128] = base[m * 128: m * 128 + 64]
    return p


def _k_perm(swap):
    """kv columns (128 = 2 groups x 64): per group block [evens|odds];
    swap=True puts group1 first (branch-2 layout)."""
    p = []
    groups = (1, 0) if swap else (0, 1)
    for g in groups:
        blk = g * D
        p += [blk + 2 * j for j in range(D // 2)]
        p += [blk + 2 * j + 1 for j in range(D // 2)]
    return np.array(p, dtype=np.int64)


def _tile_w(w, kdim, mdim):
    """(kdim*128, mdim*TS) -> (kdim, mdim, 128, TS) contiguous tiles."""
    ts = w.shape[1] // mdim
    return np.ascontiguousarray(
        w.reshape(kdim, 128, mdim, ts).transpose(0, 2, 1, 3))


def _trig_tables(core):
    pos = (256 * core + np.arange(SEQ, dtype=np.float64))  # global positions
    div = np.exp(np.arange(0, D, 2, dtype=np.float64) * (-np.log(10000.0) / D))
    ang = pos[None, :] * div[:, None]          # (32, SEQ)
    c32 = np.cos(ang).astype(np.float32)
    s32 = np.sin(ang).astype(np.float32)
    tc = np.tile(c32, (4, 1))                   # (128, SEQ)
    # sign-folded sin: rows [0:32]=+sin (qe*sin for the odd half),
    # [32:64]=-sin (-qo*sin for the even half), repeating per 64-block.
    tsn = np.tile(np.concatenate([s32, -s32], axis=0), (2, 1))
    return np.ascontiguousarray(tc), np.ascontiguousarray(tsn)


def _p32():
    """[128,128] permutation: swaps 32-halves within each 64-block.
    Used as matmul lhsT: out = P.T @ u with P[k, m] = 1 iff k = swap(m)."""
    p = np.zeros((128, 128), np.float32)
    for m in range(128):
        k = m + 32 if (m % 64) < 32 else m - 32
        p[k, m] = 1.0
    return p


# ------------------------------------------------------------ device program

_PROGRAM_CACHE = {}


def _build_program():
    import concourse.bass as bass
    import concourse.mybir as mybir
    import concourse.tile as tile
    from concourse.masks import make_identity
    from concourse.tile_rust import add_dep_helper

    def order_group(insts):
        """PE-order a bank-packed accumulation group: first (start=True)
        before everything, last (stop=True) after everything. sync=False —
        same-engine ordering only."""
        first, last = insts[0], insts[-1]
        for i in insts[1:]:
            add_dep_helper(i.ins, first.ins, sync=False,
                           reason="psum group start first")
        for i in insts[:-1]:
            add_dep_helper(last.ins, i.ins, sync=False,
                           reason="psum group stop last")

    f32 = mybir.dt.float32
    f32r = mybir.dt.float32r
    bf16 = mybir.dt.bfloat16
    ALU = mybir.AluOpType
    ACTF = mybir.ActivationFunctionType

    nc = bass.Bass()

    xt_d = nc.dram_tensor("xt", [B, 8, 128, SEQ], bf16, kind="ExternalInput")
    # wq pre-chunked on host: [chunk=(mat,mt//2), 128p, (mt%2, kt, 128)]
    wq_d = nc.dram_tensor("wq", [8, 128, 2048], bf16, kind="ExternalInput")
    wk_d = nc.dram_tensor("wk", [2, 8, 128, 128], bf16, kind="ExternalInput")
    wv_d = nc.dram_tensor("wv", [8, 128, 128], bf16, kind="ExternalInput")
    wo_d = nc.dram_tensor("wo", [8, 2, 128, 512], bf16, kind="ExternalInput")
    tc_d = nc.dram_tensor("tct", [128, SEQ], bf16, kind="ExternalInput")
    ts_d = nc.dram_tensor("tst", [128, SEQ], bf16, kind="ExternalInput")
    lam_d = nc.dram_tensor("lamv", [128, 2], f32, kind="ExternalInput")
    bout_d = nc.dram_tensor("boutv", [1, E], f32r, kind="ExternalInput")
    p32_d = nc.dram_tensor("p32", [128, 128], bf16, kind="ExternalInput")
    ones_d = nc.dram_tensor("onesv", [1, 128], f32r, kind="ExternalInput")
    y_d = nc.dram_tensor("y", [B, QROWS, E], f32, kind="ExternalOutput")

    def split_matmul_waits():
        """This walrus build allows only ONE sync-wait per engine
        instruction; peel extra waits onto engine-matched no-ops placed
        just before the instruction."""
        for bb in nc.m.functions[0].blocks:
            il = bb.instructions
            new_list = []
            changed = False
            for i in il:
                si = getattr(i, "sync_info", None)
                if si is not None and len(si.on_wait) > 1:
                    waits = list(si.on_wait)
                    for j, w in enumerate(waits[1:]):
                        nop = mybir.InstNoOp(
                            name=f"{i.name}-wnop{j}", engine=i.engine, ins=[],
                            outs=[],
                            sync_info=mybir.SyncInfo(on_wait=[w],
                                                     on_update=[]))
                        nc.inst_map[nop.name] = nop
                        new_list.append(nop)
                    i.sync_info = mybir.SyncInfo(
                        on_wait=[waits[0]], on_update=list(si.on_update))
                    changed = True
                new_list.append(i)
            if changed:
                il[:] = new_list

    with tile.TileContext(nc) as tc:
        with tc.tile_pool(name="const", bufs=1) as constp, \
             tc.tile_pool(name="xt", bufs=1) as xtp, \
             tc.tile_pool(name="rot", bufs=1) as rotp, \
             tc.tile_pool(name="wres", bufs=1) as wresp, \
             tc.tile_pool(name="ru", bufs=3) as rup, \
             tc.tile_pool(name="vext", bufs=1) as vxp, \
             tc.tile_pool(name="att", bufs=4) as attp, \
             tc.tile_pool(name="small", bufs=4) as smp, \
             tc.tile_pool(name="pairs", bufs=1) as pairp, \
             tc.tile_pool(name="atile", bufs=1) as atp, \
             tc.tile_pool(name="ysb", bufs=3) as ysbp, \
             tc.tile_pool(name="psSC", bufs=4, space="PSUM") as psSC, \
             tc.tile_pool(name="psPV", bufs=2, space="PSUM") as psPV, \
             tc.tile_pool(name="psA", bufs=2, space="PSUM") as psA:

            # ---- input DMAs (single sync queue; order = arrival order) ----
            xts = {}
            xstrips = []
            for b in range(B):
                strip = xtp.tile([128, 8 * SEQ], bf16, tag=f"xt{b}",
                                 name=f"xt{b}")
                xstrips.append(strip)
                for kt in range(8):
                    xts[b, kt] = strip[:, kt * SEQ:(kt + 1) * SEQ]

            def dma_x(b):
                nc.sync.dma_start(
                    out=xstrips[b].rearrange("p (k s) -> p k s", k=8),
                    in_=xt_d[b, :, :, :].rearrange("k p s -> p k s"))

            dma_x(0)
            wkr = wresp.tile([128, 2048], bf16, tag="wkr", name="wkr")
            nc.sync.dma_start(
                out=wkr.rearrange("p (m k s) -> p m k s", m=2, k=8),
                in_=wk_d[:, :, :, :].rearrange("m k p s -> p m k s"))
            wvr = wresp.tile([128, 1024], bf16, tag="wvr", name="wvr")
            nc.sync.dma_start(
                out=wvr.rearrange("p (k s) -> p k s", k=8),
                in_=wv_d[:, :, :].rearrange("k p s -> p k s"))

            tc_sb = constp.tile([128, SEQ], bf16, tag="tcs", name="tc_sb")
            ts_sb = constp.tile([128, SEQ], bf16, tag="tss", name="ts_sb")
            p32_sb = constp.tile([128, 128], bf16, tag="p32s", name="p32_sb")
            lam_sb = constp.tile([128, 2], f32, tag="lams", name="lam_sb")
            bout_sb = constp.tile([1, E], f32r, tag="bouts", name="bout_sb")
            ones1 = constp.tile([1, 128], f32r, tag="ones1", name="ones1")
            nc.sync.dma_start(out=tc_sb, in_=tc_d[:, :])
            nc.sync.dma_start(out=ts_sb, in_=ts_d[:, :])
            nc.sync.dma_start(out=p32_sb, in_=p32_d[:, :])
            nc.sync.dma_start(out=lam_sb, in_=lam_d[:, :])
            nc.sync.dma_start(out=bout_sb, in_=bout_d[:, :])
            nc.sync.dma_start(out=ones1, in_=ones_d[:, :])

            wqr = wresp.tile([128, 16384], bf16, tag="wqr", name="wqr")

            def dma_wq(chunk):
                nc.sync.dma_start(
                    out=wqr[:, chunk * 2048:(chunk + 1) * 2048],
                    in_=wq_d[chunk, :, :])

            # chunk = mat*4 + mt//2; order so both mats of each mt-pair
            # arrive together, in mt order.
            for chunk in (0, 4, 1, 5, 2, 6, 3, 7):
                dma_wq(chunk)
            dma_x(1)
            wor = wresp.tile([128, 8192], bf16, tag="wor", name="wor")
            nc.sync.dma_start(
                out=wor.rearrange("p (k n s) -> p k n s", k=8, n=2),
                in_=wo_d[:, :, :, :].rearrange("k n p s -> p k n s"))
            wo_sb = {}
            for kt in range(8):
                for nh in range(2):
                    off = kt * 1024 + nh * 512
                    wo_sb[kt, nh] = wor[:, off:off + 512]

            def wq_sb(mat, mt, kt):
                off = (mat * 4 + mt // 2) * 2048 + (mt % 2) * 1024 + kt * 128
                return wqr[:, off:off + 128]

            identf = constp.tile([128, 128], f32, tag="identf", name="identf")
            make_identity(nc, identf)
            identb = constp.tile([128, 128], bf16, tag="identb", name="identb")
            nc.vector.tensor_copy(identb, identf)

            # ---- shared state ----
            qrot, krot, vext, pairs, at2s = {}, {}, {}, {}, {}
            bias_sb = constp.tile([128, E], f32, tag="biasbc", name="bias_sb")

            def rope(psum_in, rot_out, name):
                # rot = qp*TC + P32 @ (qp*TS_signed); ACT casts the psum to
                # bf16 SBUF so GPSIMD (which cannot touch PSUM) does both
                # trig muls; the final add (PSUM in1) stays on DVE.
                qps = rup.tile([128, SEQ], bf16, tag="ropec",
                               name=f"c_{name}")
                nc.scalar.activation(qps, psum_in, ACTF.Copy)
                t = rup.tile([128, SEQ], bf16, tag="ropet", name=f"t_{name}")
                u = rup.tile([128, SEQ], bf16, tag="ropeu", name=f"u_{name}")
                nc.gpsimd.tensor_mul(t, qps, tc_sb)
                nc.gpsimd.tensor_mul(u, qps, ts_sb)
                uswt = psA.tile([128, 512], f32, tag="A", name=f"usw_{name}")
                usw = uswt[:, 0:SEQ]
                nc.tensor.matmul(usw, p32_sb, u, start=True, stop=True)
                nc.vector.tensor_add(rot_out, t, usw)

            def qproj_rope(b, mt):
                for mat in range(2):
                    ps = psA.tile([128, 512], f32, tag="A",
                                  name=f"qp{b}_{mat}_{mt}")
                    qp = ps[:, 0:SEQ]
                    for kt in range(8):
                        nc.tensor.matmul(
                            qp, wq_sb(mat, mt, kt), xts[b, kt],
                            start=(kt == 0), stop=(kt == 7))
                    rot = rotp.tile([128, SEQ], bf16, tag=f"q{mat}_{b}_{mt}",
                                    name=f"qr{mat}_{b}_{mt}")
                    rope(qp, rot, f"q{mat}_{b}_{mt}")
                    qrot[mat, b, mt] = rot

            def kv_prologue(b):
                for mat in range(2):
                    ps = psA.tile([128, 512], f32, tag="A",
                                  name=f"kp{b}_{mat}")
                    kp = ps[:, 0:SEQ]
                    for kt in range(8):
                        nc.tensor.matmul(
                            kp, wkr[:, mat * 1024 + kt * 128:
                                    mat * 1024 + kt * 128 + 128],
                            xts[b, kt],
                            start=(kt == 0), stop=(kt == 7))
                    rot = rotp.tile([128, SEQ], bf16, tag=f"k{mat}_{b}",
                                    name=f"kr{mat}_{b}")
                    rope(kp, rot, f"k{mat}_{b}")
                    krot[mat, b] = rot
                # v^T at full rate, then PE-transpose back to [seq, kv]
                ps = psA.tile([128, 512], f32, tag="A", name=f"vp{b}")
                vtp = ps[:, 0:SEQ]
                for kt in range(8):
                    nc.tensor.matmul(
                        vtp, wvr[:, kt * 128:(kt + 1) * 128],
                        xts[b, kt],
                        start=(kt == 0), stop=(kt == 7))
                vt_sb = rup.tile([128, SEQ], bf16, tag="vtsb",
                                 name=f"vt_sb{b}")
                nc.vector.tensor_copy(vt_sb, vtp)
                for st in range(3):
                    vtr = psA.tile([128, 256], bf16, tag="A",
                                   name=f"vtr{b}_{st}")
                    nc.tensor.matmul(vtr[:, 0:128],
                                     vt_sb[:, st * 128:(st + 1) * 128],
                                     identb, is_transpose=True)
                    for g in range(2):
                        # vext layout: [1/lam | v(64) | 1]; branch0 rhs =
                        # cols 1:66 ([v|1]), branch1 rhs = cols 0:65
                        # ([1/lam|v]).
                        ve = vxp.tile([128, 66], bf16, tag=f"ve_{b}_{st}_{g}",
                                      name=f"ve{b}_{st}_{g}")
                        nc.vector.tensor_copy(ve[:, 1:65],
                                              vtr[:, g * 64:(g + 1) * 64])
                        nc.gpsimd.tensor_copy(ve[:, 0:1], lam_sb[:, 0:1])
                        nc.gpsimd.tensor_copy(ve[:, 65:66], lam_sb[:, 1:2])
                        vext[b, st, g] = ve

            def bias_broadcast():
                for nh in range(2):
                    bps = psA.tile([128, 512], f32, tag="A", name=f"bps{nh}")
                    nc.tensor.matmul(bps, ones1,
                                     bout_sb[:, nh * 512:(nh + 1) * 512],
                                     start=True, stop=True)
                    nc.vector.tensor_copy(
                        bias_sb[:, nh * 512:(nh + 1) * 512], bps)

            # ---- attention stages (software-pipelined per batch) ----

            def stage_A(b, i, hw, ctx):
                mt, h, w = hw
                g = h & 1
                base1 = 64 * g
                base2 = 64 - base1
                scs = []
                for br, qb in ((0, base1), (1, base2)):
                    st_ps = psSC.tile([128, 512], f32, tag="sc",
                                      name=f"sc{b}_{i}_{br}")
                    mms = []
                    for kts in range(2):
                        mms.append(nc.tensor.matmul(
                            st_ps[:, kts * 256:(kts + 1) * 256],
                            krot[br, b][qb:qb + 64,
                                        w * 128 + kts * 128:
                                        w * 128 + kts * 128 + 128],
                            qrot[br, b, mt][qb:qb + 64,
                                            w * 128:w * 128 + 256],
                            start=(kts == 0), stop=(kts == 1)))
                    order_group(mms)
                    scs.append(st_ps)
                ctx[i] = {"sc": scs}

            def stage_B(b, i, hw, ctx):
                es = []
                for br in range(2):
                    e = attp.tile([128, 512], bf16, tag=f"e{br}",
                                  name=f"e{b}_{i}_{br}")
                    nc.scalar.activation(e, ctx[i]["sc"][br], ACTF.Exp,
                                         scale=0.125)
                    es.append(e)
                ctx[i]["e"] = es

            def stage_CD(b, i, hw, ctx):
                mt, h, w = hw
                g = h & 1
                es = ctx[i]["e"]
                pv = psPV.tile([128, 260], f32, tag="pv", name=f"pv{b}_{i}")
                mms = []
                first = True
                for kts in range(2):
                    for qt in range(2):
                        for br in range(2):
                            col = qt * 130 + br * 65
                            ve = vext[b, w + kts, g]
                            rhs = ve[:, 0:65] if br == 1 else ve[:, 1:66]
                            mms.append(nc.tensor.matmul(
                                pv[:, col:col + 65],
                                es[br][:, kts * 256 + qt * 128:
                                       kts * 256 + qt * 128 + 128],
                                rhs,
                                start=first,
                                stop=(kts == 1 and qt == 1 and br == 1)))
                            first = False
                order_group(mms)

                # denominators: cols 64,65 (qt0: r1, lam*r2) and 194,195
                r_sb = smp.tile([128, 4], f32, tag="recip", name=f"r{b}_{i}")
                nc.vector.reciprocal(
                    r_sb.rearrange("p (c k) -> p c k", c=2),
                    pv.rearrange("p (c k) -> p c k", c=2)[:, :, 64:66])
                if g == 0:
                    for qt in range(2):
                        pairs[b, mt, w, qt] = pairp.tile(
                            [128, 128], bf16, tag=f"pair{b}_{mt}_{w}_{qt}",
                            name=f"pair{b}_{mt}_{w}_{qt}")
                for qt in range(2):
                    t1 = smp.tile([128, 64], bf16, tag=f"t1_{qt}",
                                  name=f"t1_{b}_{i}_{qt}")
                    nc.vector.tensor_scalar_mul(
                        t1, pv[:, qt * 130:qt * 130 + 64],
                        r_sb[:, 2 * qt:2 * qt + 1])
                    # pair = (pv_br2 * lam*r2) - pv_br1*r1 = -(a); Wout is
                    # negated on the host to compensate.
                    nc.vector.scalar_tensor_tensor(
                        out=pairs[b, mt, w, qt][:, g * 64:(g + 1) * 64],
                        in0=pv[:, qt * 130 + 66:qt * 130 + 130],
                        scalar=r_sb[:, 2 * qt + 1:2 * qt + 2],
                        in1=t1, op0=ALU.mult, op1=ALU.subtract)

            def stage_E(b, mt, w):
                tr = psA.tile([128, 256], bf16, tag="A", name=f"tr{b}_{mt}_{w}")
                for qt in range(2):
                    nc.tensor.transpose(tr[:, qt * 128:(qt + 1) * 128],
                                        pairs[b, mt, w, qt], identb)
                at = atp.tile([128, 256], bf16, tag=f"at{b}_{w}_{mt}",
                              name=f"at{b}_{w}_{mt}")
                nc.vector.tensor_copy(at, tr)
                at2s[b, w, mt] = at

            def run_pipeline(b):
                hws = [(mt, 2 * mt + gg, w)
                       for mt in range(8) for gg in (0, 1) for w in (0, 1)]
                n = len(hws)
                ctx = {}
                for i in range(n + 2):
                    if i < n:
                        mt, h, w = hws[i]
                        if h == 2 * mt and w == 0 and mt + 2 < 8:
                            qproj_rope(b, mt + 2)
                        stage_A(b, i, hws[i], ctx)
                    if 0 <= i - 1 < n:
                        stage_B(b, i - 1, hws[i - 1], ctx)
                    if 0 <= i - 2 < n:
                        j = i - 2
                        stage_CD(b, j, hws[j], ctx)
                        mt, h, w = hws[j]
                        if h % 2 == 1:
                            stage_E(b, mt, w)
                        del ctx[j]

            def outproj(b, w):
                for qt in range(2):
                    for nh in range(2):
                        y_ps = psA.tile([128, 512], f32, tag="A",
                                        name=f"y{b}_{w}_{qt}_{nh}")
                        mms = []
                        for kt in range(8):
                            mms.append(nc.tensor.matmul(
                                y_ps,
                                at2s[b, w, kt][:, qt * 128:(qt + 1) * 128],
                                wo_sb[kt, nh],
                                start=(kt == 0), stop=(kt == 7)))
                        order_group(mms)
                        y_sb = ysbp.tile([128, 512], f32, tag="ysb",
                                         name=f"ysb{b}_{w}_{qt}_{nh}")
                        nc.vector.tensor_add(
                            y_sb, y_ps,
                            bias_sb[:, nh * 512:(nh + 1) * 512])
                        nc.gpsimd.dma_start(
                            out=y_d[b, (w * 2 + qt) * 128:
                                    (w * 2 + qt) * 128 + 128,
                                    nh * 512:(nh + 1) * 512],
                            in_=y_sb)

            # ---- emission schedule ----
            kv_prologue(0)
            bias_broadcast()
            qproj_rope(0, 0)
            qproj_rope(0, 1)
            run_pipeline(0)
            kv_prologue(1)
            qproj_rope(1, 0)
            qproj_rope(1, 1)
            outproj(0, 0)
            outproj(0, 1)
            run_pipeline(1)
            outproj(1, 0)
            outproj(1, 1)

    split_matmul_waits()
    return nc


def get_program():
    if "nc" not in _PROGRAM_CACHE:
        _PROGRAM_CACHE["nc"] = _build_program()
    return _PROGRAM_CACHE["nc"]


# ------------------------------------------------------------------ host API

def make_in_maps(x, Wq1, Wq2, Wk1, Wk2, Wv, Wout, bout, lq1, lk1, lq2, lk2):
    import ml_dtypes
    bf16 = ml_dtypes.bfloat16

    x = np.asarray(x, dtype=np.float32)
    lam = float(np.clip(
        np.exp(np.asarray(lq1, np.float64) @ np.asarray(lk1, np.float64))
        - np.exp(np.asarray(lq2, np.float64) @ np.asarray(lk2, np.float64))
        + LAMBDA_INIT, 0.1, 0.9))

    qp1, qp2 = _head_perm(), _q2_perm()
    kp1, kp2 = _k_perm(False), _k_perm(True)

    wq_t = np.stack([
        _tile_w(np.asarray(Wq1, np.float32)[:, qp1], 8, 8),
        _tile_w(np.asarray(Wq2, np.float32)[:, qp2], 8, 8),
    ])  # (mat, kt, mt, 128, 128)
    # chunk layout for single contiguous DMAs:
    # (mat, mt//2, p, mt%2, kt, s) -> (8, 128, 2048)
    wq = np.ascontiguousarray(
        wq_t.reshape(2, 8, 4, 2, 128, 128)
        .transpose(0, 2, 4, 3, 1, 5).reshape(8, 128, 2048)).astype(bf16)
    wk = np.stack([
        _tile_w(np.asarray(Wk1, np.float32)[:, kp1], 8, 1)[:, 0],
        _tile_w(np.asarray(Wk2, np.float32)[:, kp2], 8, 1)[:, 0],
    ]).astype(bf16)  # (2, 8, 128, 128)
    wv = _tile_w(np.asarray(Wv, np.float32), 8, 1)[:, 0].astype(bf16)
    # NOTE: negated — the device computes -a (branch2-scaled minus branch1).
    wo = _tile_w(-np.asarray(Wout, np.float32), 8, 2).astype(bf16)
    boutv = np.asarray(bout, np.float32).reshape(1, E)

    lamv = np.zeros((128, 2), np.float32)
    lamv[:, 0] = 1.0 / lam  # branch-2 "ones" column -> den2/lam
    lamv[:, 1] = 1.0        # branch-1 ones column

    # x^T, tiled: (B, 8, 128, SEQ) per core
    xT = np.ascontiguousarray(x.transpose(0, 2, 1))  # (B, E, L)

    in_maps = []
    for c in range(NCORES):
        s0 = 256 * c
        xt = np.ascontiguousarray(
            xT[:, :, s0:s0 + SEQ].reshape(B, 8, 128, SEQ)).astype(bf16)
        tct, tst = _trig_tables(c)
        in_maps.append({
            "xt": xt, "wq": wq, "wk": wk, "wv": wv, "wo": wo,
            "tct": tct.astype(bf16), "tst": tst.astype(bf16),
            "lamv": lamv, "boutv": boutv,
            "p32": _p32().astype(bf16),
            "onesv": np.ones((1, 128), np.float32),
        })
    return in_maps


def kernel(**inputs) -> np.ndarray:
    from concourse.bass_utils import run_bass_kernel_spmd

    in_maps = make_in_maps(**inputs)
    nc = get_program()
    res = run_bass_kernel_spmd(nc, in_maps, core_ids=list(range(NCORES)))
    out = np.empty((B, L, E), dtype=np.float32)
    for c in range(NCORES):
        out[:, 512 * c:512 * (c + 1), :] = res.results[c]["y"]
    return out


# revision 17
# speedup vs baseline: 1.3068x; 1.3068x over previous
"""Trainium2 Bass kernel for EnhancedLocalAttentionWithGQA (differential
windowed attention, B=2 L=4096 E=1024 H=16 G=2 W=256 D=64).

Structural facts exploited (same as prior version):
  - Only windows 0..15 contribute; core c owns windows (2c, 2c+1) ->
    output rows [512c, 512c+512) per batch; needs x rows [256c, 256c+384).
  - q^T/k^T computed in [head-dim, seq] layout with host-permuted weight
    columns so RoPE is 2 tensor muls + a P32 permutation matmul + add.
  - Branch-2 weights block-swapped so the two differential branches use
    complementary 64-partition halves (K=64 score matmuls).
  - Scores transposed S^T[k, q]; exp without max-subtraction; PV with an
    extra ones column giving the softmax denominators.

This version restructures for engine balance + pipelining:
  - ACT does ONLY the exps (plus cheap at-tile copies); the old
    ACT-copy-with-scale normalize step is replaced by a DVE/GPSIMD
    tensor_scalar + scalar_tensor_tensor pair. The lambda fold is baked
    into the PV rhs: vext = [1/lam | v | 1], so branch-2's denominator
    column comes out pre-divided by lambda and one reciprocal yields
    both branch scales. The combined sign flip (pair = lam*a2' - a1') is
    fixed by negating Wout on the host.
  - RoPE: t-mul + final add on DVE, u-mul on GPSIMD, P32 swap on PE.
  - Software pipelining: per batch, head-group mt's attention stages are
    emitted with a 2-stage lag (scores -> exp -> PV+normalize) and
    q-projections for mt+2 are emitted between stages; batch-1
    projections interleave with batch-0's out-projection so the PE never
    idles (stays at 2.4 GHz).
"""

import os
import sys

sys.path.insert(0, "/opt/trn_rl_repo")
os.environ.setdefault("MYCRO_LOCAL_CACHE", "1")

import numpy as np

B, L, E, H, G, W, D = 2, 4096, 1024, 16, 2, 256, 64
NCORES = 8
SEQ = 384          # x rows per core
NW = 2             # windows per core
QROWS = 512        # output rows per core per batch
KV = E // (H // G)  # 128
LAMBDA_INIT = 0.8


# ----------------------------------------------------------------- host prep

def _head_perm():
    """Column permutation applied to Wq1/Wk1: per 64-block -> [evens|odds]."""
    p = []
    for blk in range(0, E, D):
        p += [blk + 2 * j for j in range(D // 2)]
        p += [blk + 2 * j + 1 for j in range(D // 2)]
    return np.array(p, dtype=np.int64)


def _q2_perm():
    """q2: like _head_perm but heads swapped within each 128-col M-tile."""
    base = _head_perm()
    p = np.empty_like(base)
    for m in range(E // 128):
        p[m * 128: m * 128 + 64] = base[m * 128 + 64: m * 128 + 128]
        p[m * 128 + 64: m * 128 + 128] = base[m * 128: m * 128 + 64]
    return p


def _k_perm(swap):
    """kv columns (128 = 2 groups x 64): per group block [evens|odds];
    swap=True puts group1 first (branch-2 layout)."""
    p = []
    groups = (1, 0) if swap else (0, 1)
    for g in groups:
        blk = g * D
        p += [blk + 2 * j for j in range(D // 2)]
        p += [blk + 2 * j + 1 for j in range(D // 2)]
    return np.array(p, dtype=np.int64)


def _tile_w(w, kdim, mdim):
    """(kdim*128, mdim*TS) -> (kdim, mdim, 128, TS) contiguous tiles."""
    ts = w.shape[1] // mdim
    return np.ascontiguousarray(
        w.reshape(kdim, 128, mdim, ts).transpose(0, 2, 1, 3))


def _trig_tables(core):
    pos = (256 * core + np.arange(SEQ, dtype=np.float64))  # global positions
    div = np.exp(np.arange(0, D, 2, dtype=np.float64) * (-np.log(10000.0) / D))
    ang = pos[None, :] * div[:, None]          # (32, SEQ)
    c32 = np.cos(ang).astype(np.float32)
    s32 = np.sin(ang).astype(np.float32)
    tc = np.tile(c32, (4, 1))                   # (128, SEQ)
    # sign-folded sin: rows [0:32]=+sin (qe*sin for the odd half),
    # [32:64]=-sin (-qo*sin for the even half), repeating per 64-block.
    tsn = np.tile(np.concatenate([s32, -s32], axis=0), (2, 1))
    return np.ascontiguousarray(tc), np.ascontiguousarray(tsn)


def _p32():
    """[128,128] permutation: swaps 32-halves within each 64-block.
    Used as matmul lhsT: out = P.T @ u with P[k, m] = 1 iff k = swap(m)."""
    p = np.zeros((128, 128), np.float32)
    for m in range(128):
        k = m + 32 if (m % 64) < 32 else m - 32
        p[k, m] = 1.0
    return p


# ------------------------------------------------------------ device program

_PROGRAM_CACHE = {}


def _build_program():
    import concourse.bass as bass
    import concourse.mybir as mybir
    import concourse.tile as tile
    from concourse.masks import make_identity
    from concourse.tile_rust import add_dep_helper

    def order_group(insts):
        """PE-order a bank-packed accumulation group: first (start=True)
        before everything, last (stop=True) after everything. sync=False —
        same-engine ordering only."""
        first, last = insts[0], insts[-1]
        for i in insts[1:]:
            add_dep_helper(i.ins, first.ins, sync=False,
                           reason="psum group start first")
        for i in insts[:-1]:
            add_dep_helper(last.ins, i.ins, sync=False,
                           reason="psum group stop last")

    f32 = mybir.dt.float32
    f32r = mybir.dt.float32r
    bf16 = mybir.dt.bfloat16
    ALU = mybir.AluOpType
    ACTF = mybir.ActivationFunctionType

    nc = bass.Bass()

    # All inputs partition-major and contiguous per partition row so each
    # DMA is a single max-bandwidth linear transfer.
    xt_d = nc.dram_tensor("xt", [128, B * 8 * SEQ], bf16,
                          kind="ExternalInput")
    # wq pre-chunked on host: [chunk=(mat,mt//2), 128p, (mt%2, kt, 128)]
    wq_d = nc.dram_tensor("wq", [8, 128, 2048], bf16, kind="ExternalInput")
    wk_d = nc.dram_tensor("wk", [128, 2048], bf16, kind="ExternalInput")
    wv_d = nc.dram_tensor("wv", [128, 1024], bf16, kind="ExternalInput")
    wo_d = nc.dram_tensor("wo", [128, 8192], bf16, kind="ExternalInput")
    tc_d = nc.dram_tensor("tct", [128, SEQ], bf16, kind="ExternalInput")
    ts_d = nc.dram_tensor("tst", [128, SEQ], bf16, kind="ExternalInput")
    lam_d = nc.dram_tensor("lamv", [128, 2], f32, kind="ExternalInput")
    bout_d = nc.dram_tensor("boutv", [1, E], f32r, kind="ExternalInput")
    p32_d = nc.dram_tensor("p32", [128, 128], bf16, kind="ExternalInput")
    ones_d = nc.dram_tensor("onesv", [1, 128], f32r, kind="ExternalInput")
    y_d = nc.dram_tensor("y", [B, QROWS, E], f32, kind="ExternalOutput")

    def split_matmul_waits():
        """This walrus build allows only ONE sync-wait per engine
        instruction; peel extra waits onto engine-matched no-ops placed
        just before the instruction."""
        for bb in nc.m.functions[0].blocks:
            il = bb.instructions
            new_list = []
            changed = False
            for i in il:
                si = getattr(i, "sync_info", None)
                if si is not None and len(si.on_wait) > 1:
                    waits = list(si.on_wait)
                    for j, w in enumerate(waits[1:]):
                        nop = mybir.InstNoOp(
                            name=f"{i.name}-wnop{j}", engine=i.engine, ins=[],
                            outs=[],
                            sync_info=mybir.SyncInfo(on_wait=[w],
                                                     on_update=[]))
                        nc.inst_map[nop.name] = nop
                        new_list.append(nop)
                    i.sync_info = mybir.SyncInfo(
                        on_wait=[waits[0]], on_update=list(si.on_update))
                    changed = True
                new_list.append(i)
            if changed:
                il[:] = new_list

    with tile.TileContext(nc) as tc:
        with tc.tile_pool(name="const", bufs=1) as constp, \
             tc.tile_pool(name="xt", bufs=1) as xtp, \
             tc.tile_pool(name="rot", bufs=1) as rotp, \
             tc.tile_pool(name="wres", bufs=1) as wresp, \
             tc.tile_pool(name="ru", bufs=3) as rup, \
             tc.tile_pool(name="vext", bufs=1) as vxp, \
             tc.tile_pool(name="att", bufs=4) as attp, \
             tc.tile_pool(name="small", bufs=4) as smp, \
             tc.tile_pool(name="pairs", bufs=1) as pairp, \
             tc.tile_pool(name="atile", bufs=1) as atp, \
             tc.tile_pool(name="ysb", bufs=3) as ysbp, \
             tc.tile_pool(name="psSC", bufs=4, space="PSUM") as psSC, \
             tc.tile_pool(name="psPV", bufs=2, space="PSUM") as psPV, \
             tc.tile_pool(name="psA", bufs=2, space="PSUM") as psA:

            # ---- input DMAs: two queues in parallel. sync: batch-0 x +
            # k/v weights + consts (everything the prologue needs, ~1.3MB);
            # gpsimd: wq + batch-1 x + wo (6.8MB, needed later).
            xts = {}
            xstrip = xtp.tile([128, B * 8 * SEQ], bf16, tag="xt",
                              name="xstrip")
            for b in range(B):
                for kt in range(8):
                    off = (b * 8 + kt) * SEQ
                    xts[b, kt] = xstrip[:, off:off + SEQ]

            nc.sync.dma_start(out=xstrip[:, 0:8 * SEQ],
                              in_=xt_d[:, 0:8 * SEQ])
            wkr = wresp.tile([128, 2048], bf16, tag="wkr", name="wkr")
            nc.sync.dma_start(out=wkr, in_=wk_d[:, :])
            wvr = wresp.tile([128, 1024], bf16, tag="wvr", name="wvr")
            nc.sync.dma_start(out=wvr, in_=wv_d[:, :])

            tc_sb = constp.tile([128, SEQ], bf16, tag="tcs", name="tc_sb")
            ts_sb = constp.tile([128, SEQ], bf16, tag="tss", name="ts_sb")
            p32_sb = constp.tile([128, 128], bf16, tag="p32s", name="p32_sb")
            lam_sb = constp.tile([128, 2], f32, tag="lams", name="lam_sb")
            bout_sb = constp.tile([1, E], f32r, tag="bouts", name="bout_sb")
            ones1 = constp.tile([1, 128], f32r, tag="ones1", name="ones1")
            nc.sync.dma_start(out=tc_sb, in_=tc_d[:, :])
            nc.sync.dma_start(out=ts_sb, in_=ts_d[:, :])
            nc.sync.dma_start(out=p32_sb, in_=p32_d[:, :])
            nc.sync.dma_start(out=lam_sb, in_=lam_d[:, :])
            nc.sync.dma_start(out=bout_sb, in_=bout_d[:, :])
            nc.sync.dma_start(out=ones1, in_=ones_d[:, :])

            wqr = wresp.tile([128, 16384], bf16, tag="wqr", name="wqr")

            # chunk = mat*4 + mt//2; order so both mats of each mt-pair
            # arrive together, in mt order.
            for chunk in (0, 4, 1, 5, 2, 6, 3, 7):
                nc.gpsimd.dma_start(
                    out=wqr[:, chunk * 2048:(chunk + 1) * 2048],
                    in_=wq_d[chunk, :, :])
            nc.gpsimd.dma_start(out=xstrip[:, 8 * SEQ:16 * SEQ],
                                in_=xt_d[:, 8 * SEQ:16 * SEQ])
            wor = wresp.tile([128, 8192], bf16, tag="wor", name="wor")
            nc.gpsimd.dma_start(out=wor, in_=wo_d[:, :])
            wo_sb = {}
            for kt in range(8):
                for nh in range(2):
                    off = kt * 1024 + nh * 512
                    wo_sb[kt, nh] = wor[:, off:off + 512]

            def wq_sb(mat, mt, kt):
                off = (mat * 4 + mt // 2) * 2048 + (mt % 2) * 1024 + kt * 128
                return wqr[:, off:off + 128]

            identf = constp.tile([128, 128], f32, tag="identf", name="identf")
            make_identity(nc, identf)
            identb = constp.tile([128, 128], bf16, tag="identb", name="identb")
            nc.vector.tensor_copy(identb, identf)

            # ---- shared state ----
            qrot, krot, vext, pairs, at2s = {}, {}, {}, {}, {}
            bias_sb = constp.tile([128, E], f32, tag="biasbc", name="bias_sb")

            # RoPE is split so its 4-engine chain never head-of-line blocks
            # the PE queue: the cast (ACT, frees the proj psum slot) is
            # emitted right after the projection matmuls; the rot part (GPS
            # muls -> PE P32 -> DVE add) is deferred until other PE work has
            # been queued.
            def rope_cast(psum_in, name):
                qps = rup.tile([128, SEQ], bf16, tag="ropec",
                               name=f"c_{name}")
                nc.scalar.activation(qps, psum_in, ACTF.Copy)
                return qps

            def rope_rot(qps, rot_out, name):
                t = rup.tile([128, SEQ], bf16, tag="ropet", name=f"t_{name}")
                u = rup.tile([128, SEQ], bf16, tag="ropeu", name=f"u_{name}")
                nc.gpsimd.tensor_mul(t, qps, tc_sb)
                nc.gpsimd.tensor_mul(u, qps, ts_sb)
                uswt = psA.tile([128, 512], f32, tag="A", name=f"usw_{name}")
                usw = uswt[:, 0:SEQ]
                nc.tensor.matmul(usw, p32_sb, u, start=True, stop=True)
                nc.vector.tensor_add(rot_out, t, usw)

            def qproj_cast(b, mt):
                casts = []
                for mat in range(2):
                    ps = psA.tile([128, 512], f32, tag="A",
                                  name=f"qp{b}_{mat}_{mt}")
                    qp = ps[:, 0:SEQ]
                    for kt in range(8):
                        nc.tensor.matmul(
                            qp, wq_sb(mat, mt, kt), xts[b, kt],
                            start=(kt == 0), stop=(kt == 7))
                    casts.append(rope_cast(qp, f"q{mat}_{b}_{mt}"))
                return casts

            def qproj_finish(b, mt, casts):
                for mat in range(2):
                    rot = rotp.tile([128, SEQ], bf16, tag=f"q{mat}_{b}_{mt}",
                                    name=f"qr{mat}_{b}_{mt}")
                    rope_rot(casts[mat], rot, f"q{mat}_{b}_{mt}")
                    qrot[mat, b, mt] = rot

            def kv_cast(b):
                st = {}
                for mat in range(2):
                    ps = psA.tile([128, 512], f32, tag="A",
                                  name=f"kp{b}_{mat}")
                    kp = ps[:, 0:SEQ]
                    for kt in range(8):
                        nc.tensor.matmul(
                            kp, wkr[:, mat * 1024 + kt * 128:
                                    mat * 1024 + kt * 128 + 128],
                            xts[b, kt],
                            start=(kt == 0), stop=(kt == 7))
                    st[f"k{mat}"] = rope_cast(kp, f"k{mat}_{b}")
                # v^T at full rate
                ps = psA.tile([128, 512], f32, tag="A", name=f"vp{b}")
                vtp = ps[:, 0:SEQ]
                for kt in range(8):
                    nc.tensor.matmul(
                        vtp, wvr[:, kt * 128:(kt + 1) * 128],
                        xts[b, kt],
                        start=(kt == 0), stop=(kt == 7))
                vt_sb = rup.tile([128, SEQ], bf16, tag="vtsb",
                                 name=f"vt_sb{b}")
                nc.vector.tensor_copy(vt_sb, vtp)
                st["vt"] = vt_sb
                return st

            def kv_finish(b, st):
                for mat in range(2):
                    rot = rotp.tile([128, SEQ], bf16, tag=f"k{mat}_{b}",
                                    name=f"kr{mat}_{b}")
                    rope_rot(st[f"k{mat}"], rot, f"k{mat}_{b}")
                    krot[mat, b] = rot
                vt_sb = st["vt"]
                for s in range(3):
                    vtr = psA.tile([128, 256], bf16, tag="A",
                                   name=f"vtr{b}_{s}")
                    nc.tensor.matmul(vtr[:, 0:128],
                                     vt_sb[:, s * 128:(s + 1) * 128],
                                     identb, is_transpose=True)
                    for g in range(2):
                        # vext layout: [1/lam | v(64) | 1]; branch0 rhs =
                        # cols 1:66 ([v|1]), branch1 rhs = cols 0:65
                        # ([1/lam|v]).
                        ve = vxp.tile([128, 66], bf16, tag=f"ve_{b}_{s}_{g}",
                                      name=f"ve{b}_{s}_{g}")
                        nc.vector.tensor_copy(ve[:, 1:65],
                                              vtr[:, g * 64:(g + 1) * 64])
                        nc.gpsimd.tensor_copy(ve[:, 0:1], lam_sb[:, 0:1])
                        nc.gpsimd.tensor_copy(ve[:, 65:66], lam_sb[:, 1:2])
                        vext[b, s, g] = ve

            def bias_broadcast():
                for nh in range(2):
                    bps = psA.tile([128, 512], f32, tag="A", name=f"bps{nh}")
                    nc.tensor.matmul(bps, ones1,
                                     bout_sb[:, nh * 512:(nh + 1) * 512],
                                     start=True, stop=True)
                    nc.vector.tensor_copy(
                        bias_sb[:, nh * 512:(nh + 1) * 512], bps)

            # ---- attention stages (software-pipelined per batch) ----

            def stage_A(b, i, hw, ctx):
                mt, h, w = hw
                g = h & 1
                base1 = 64 * g
                base2 = 64 - base1
                scs = []
                for br, qb in ((0, base1), (1, base2)):
                    st_ps = psSC.tile([128, 512], f32, tag="sc",
                                      name=f"sc{b}_{i}_{br}")
                    mms = []
                    for kts in range(2):
                        mms.append(nc.tensor.matmul(
                            st_ps[:, kts * 256:(kts + 1) * 256],
                            krot[br, b][qb:qb + 64,
                                        w * 128 + kts * 128:
                                        w * 128 + kts * 128 + 128],
                            qrot[br, b, mt][qb:qb + 64,
                                            w * 128:w * 128 + 256],
                            start=(kts == 0), stop=(kts == 1)))
                    order_group(mms)
                    scs.append(st_ps)
                ctx[i] = {"sc": scs}

            def stage_B(b, i, hw, ctx):
                es = []
                for br in range(2):
                    e = attp.tile([128, 512], bf16, tag=f"e{br}",
                                  name=f"e{b}_{i}_{br}")
                    nc.scalar.activation(e, ctx[i]["sc"][br], ACTF.Exp,
                                         scale=0.125)
                    es.append(e)
                ctx[i]["e"] = es

            def stage_CD(b, i, hw, ctx):
                mt, h, w = hw
                g = h & 1
                es = ctx[i]["e"]
                pv = psPV.tile([128, 260], f32, tag="pv", name=f"pv{b}_{i}")
                mms = []
                first = True
                for kts in range(2):
                    for qt in range(2):
                        for br in range(2):
                            col = qt * 130 + br * 65
                            ve = vext[b, w + kts, g]
                            rhs = ve[:, 0:65] if br == 1 else ve[:, 1:66]
                            mms.append(nc.tensor.matmul(
                                pv[:, col:col + 65],
                                es[br][:, kts * 256 + qt * 128:
                                       kts * 256 + qt * 128 + 128],
                                rhs,
                                start=first,
                                stop=(kts == 1 and qt == 1 and br == 1)))
                            first = False
                order_group(mms)

                # denominators: cols 64,65 (qt0: r1, lam*r2) and 194,195
                r_sb = smp.tile([128, 4], f32, tag="recip", name=f"r{b}_{i}")
                nc.vector.reciprocal(
                    r_sb.rearrange("p (c k) -> p c k", c=2),
                    pv.rearrange("p (c k) -> p c k", c=2)[:, :, 64:66])
                if g == 0:
                    for qt in range(2):
                        pairs[b, mt, w, qt] = pairp.tile(
                            [128, 128], bf16, tag=f"pair{b}_{mt}_{w}_{qt}",
                            name=f"pair{b}_{mt}_{w}_{qt}")
                for qt in range(2):
                    t1 = smp.tile([128, 64], bf16, tag=f"t1_{qt}",
                                  name=f"t1_{b}_{i}_{qt}")
                    nc.vector.tensor_scalar_mul(
                        t1, pv[:, qt * 130:qt * 130 + 64],
                        r_sb[:, 2 * qt:2 * qt + 1])
                    # pair = (pv_br2 * lam*r2) - pv_br1*r1 = -(a); Wout is
                    # negated on the host to compensate.
                    nc.vector.scalar_tensor_tensor(
                        out=pairs[b, mt, w, qt][:, g * 64:(g + 1) * 64],
                        in0=pv[:, qt * 130 + 66:qt * 130 + 130],
                        scalar=r_sb[:, 2 * qt + 1:2 * qt + 2],
                        in1=t1, op0=ALU.mult, op1=ALU.subtract)

            def stage_E(b, mt, w):
                tr = psA.tile([128, 256], bf16, tag="A", name=f"tr{b}_{mt}_{w}")
                for qt in range(2):
                    nc.tensor.transpose(tr[:, qt * 128:(qt + 1) * 128],
                                        pairs[b, mt, w, qt], identb)
                at = atp.tile([128, 256], bf16, tag=f"at{b}_{w}_{mt}",
                              name=f"at{b}_{w}_{mt}")
                nc.vector.tensor_copy(at, tr)
                at2s[b, w, mt] = at

            def run_pipeline(b):
                hws = [(mt, 2 * mt + gg, w)
                       for mt in range(8) for gg in (0, 1) for w in (0, 1)]
                n = len(hws)
                ctx = {}
                pend = {}
                for i in range(n + 2):
                    if i < n:
                        stage_A(b, i, hws[i], ctx)
                    if 0 <= i - 1 < n:
                        stage_B(b, i - 1, hws[i - 1], ctx)
                    if i < n:
                        # q-projection prefetch, two groups ahead; emitted
                        # after this step's exps so the ACT cast doesn't
                        # delay them.
                        mt, h, w = hws[i]
                        if h == 2 * mt and w == 0:
                            if mt + 1 in pend:
                                qproj_finish(b, mt + 1, pend.pop(mt + 1))
                            if mt + 2 < 8:
                                pend[mt + 2] = qproj_cast(b, mt + 2)
                    if 0 <= i - 2 < n:
                        j = i - 2
                        stage_CD(b, j, hws[j], ctx)
                        mt, h, w = hws[j]
                        if h % 2 == 1:
                            stage_E(b, mt, w)
                        del ctx[j]

            def outproj(b, w):
                for qt in range(2):
                    for nh in range(2):
                        y_ps = psA.tile([128, 512], f32, tag="A",
                                        name=f"y{b}_{w}_{qt}_{nh}")
                        mms = []
                        for kt in range(8):
                            mms.append(nc.tensor.matmul(
                                y_ps,
                                at2s[b, w, kt][:, qt * 128:(qt + 1) * 128],
                                wo_sb[kt, nh],
                                start=(kt == 0), stop=(kt == 7)))
                        order_group(mms)
                        y_sb = ysbp.tile([128, 512], f32, tag="ysb",
                                         name=f"ysb{b}_{w}_{qt}_{nh}")
                        nc.vector.tensor_add(
                            y_sb, y_ps,
                            bias_sb[:, nh * 512:(nh + 1) * 512])
                        dma_eng = nc.gpsimd if (qt + nh) % 2 == 0 else nc.sync
                        dma_eng.dma_start(
                            out=y_d[b, (w * 2 + qt) * 128:
                                    (w * 2 + qt) * 128 + 128,
                                    nh * 512:(nh + 1) * 512],
                            in_=y_sb)

            # ---- emission schedule ----
            st0 = kv_cast(0)
            c00 = qproj_cast(0, 0)
            c01 = qproj_cast(0, 1)
            bias_broadcast()
            kv_finish(0, st0)
            qproj_finish(0, 0, c00)
            qproj_finish(0, 1, c01)
            run_pipeline(0)
            st1 = kv_cast(1)
            c10 = qproj_cast(1, 0)
            c11 = qproj_cast(1, 1)
            outproj(0, 0)
            kv_finish(1, st1)
            qproj_finish(1, 0, c10)
            qproj_finish(1, 1, c11)
            outproj(0, 1)
            run_pipeline(1)
            outproj(1, 0)
            outproj(1, 1)

    split_matmul_waits()
    return nc


def get_program():
    if "nc" not in _PROGRAM_CACHE:
        _PROGRAM_CACHE["nc"] = _build_program()
    return _PROGRAM_CACHE["nc"]


# ------------------------------------------------------------------ host API

def make_in_maps(x, Wq1, Wq2, Wk1, Wk2, Wv, Wout, bout, lq1, lk1, lq2, lk2):
    import ml_dtypes
    bf16 = ml_dtypes.bfloat16

    x = np.asarray(x, dtype=np.float32)
    lam = float(np.clip(
        np.exp(np.asarray(lq1, np.float64) @ np.asarray(lk1, np.float64))
        - np.exp(np.asarray(lq2, np.float64) @ np.asarray(lk2, np.float64))
        + LAMBDA_INIT, 0.1, 0.9))

    qp1, qp2 = _head_perm(), _q2_perm()
    kp1, kp2 = _k_perm(False), _k_perm(True)

    wq_t = np.stack([
        _tile_w(np.asarray(Wq1, np.float32)[:, qp1], 8, 8),
        _tile_w(np.asarray(Wq2, np.float32)[:, qp2], 8, 8),
    ])  # (mat, kt, mt, 128, 128)
    # chunk layout for single contiguous DMAs:
    # (mat, mt//2, p, mt%2, kt, s) -> (8, 128, 2048)
    wq = np.ascontiguousarray(
        wq_t.reshape(2, 8, 4, 2, 128, 128)
        .transpose(0, 2, 4, 3, 1, 5).reshape(8, 128, 2048)).astype(bf16)
    # wk/wv/wo partition-major contiguous: [128, ...]
    wk = np.ascontiguousarray(np.stack([
        _tile_w(np.asarray(Wk1, np.float32)[:, kp1], 8, 1)[:, 0],
        _tile_w(np.asarray(Wk2, np.float32)[:, kp2], 8, 1)[:, 0],
    ]).transpose(2, 0, 1, 3).reshape(128, 2048)).astype(bf16)
    wv = np.ascontiguousarray(
        _tile_w(np.asarray(Wv, np.float32), 8, 1)[:, 0]
        .transpose(1, 0, 2).reshape(128, 1024)).astype(bf16)
    # NOTE: negated — the device computes -a (branch2-scaled minus branch1).
    wo = np.ascontiguousarray(
        _tile_w(-np.asarray(Wout, np.float32), 8, 2)
        .transpose(2, 0, 1, 3).reshape(128, 8192)).astype(bf16)
    boutv = np.asarray(bout, np.float32).reshape(1, E)

    lamv = np.zeros((128, 2), np.float32)
    lamv[:, 0] = 1.0 / lam  # branch-2 "ones" column -> den2/lam
    lamv[:, 1] = 1.0        # branch-1 ones column

    # x^T, tiled: (B, 8, 128, SEQ) per core
    xT = np.ascontiguousarray(x.transpose(0, 2, 1))  # (B, E, L)

    in_maps = []
    for c in range(NCORES):
        s0 = 256 * c
        xt = np.ascontiguousarray(
            xT[:, :, s0:s0 + SEQ].reshape(B, 8, 128, SEQ)
            .transpose(2, 0, 1, 3).reshape(128, B * 8 * SEQ)).astype(bf16)
        tct, tst = _trig_tables(c)
        in_maps.append({
            "xt": xt, "wq": wq, "wk": wk, "wv": wv, "wo": wo,
            "tct": tct.astype(bf16), "tst": tst.astype(bf16),
            "lamv": lamv, "boutv": boutv,
            "p32": _p32().astype(bf16),
            "onesv": np.ones((1, 128), np.float32),
        })
    return in_maps


def kernel(**inputs) -> np.ndarray:
    from concourse.bass_utils import run_bass_kernel_spmd

    in_maps = make_in_maps(**inputs)
    nc = get_program()
    res = run_bass_kernel_spmd(nc, in_maps, core_ids=list(range(NCORES)))
    out = np.empty((B, L, E), dtype=np.float32)
    for c in range(NCORES):
        out[:, 512 * c:512 * (c + 1), :] = res.results[c]["y"]
    return out


# revision 26
# speedup vs baseline: 1.3853x; 1.0600x over previous
"""Trainium2 Bass kernel for EnhancedLocalAttentionWithGQA (differential
windowed attention, B=2 L=4096 E=1024 H=16 G=2 W=256 D=64).

Structural facts exploited (same as prior version):
  - Only windows 0..15 contribute; core c owns windows (2c, 2c+1) ->
    output rows [512c, 512c+512) per batch; needs x rows [256c, 256c+384).
  - q^T/k^T computed in [head-dim, seq] layout with host-permuted weight
    columns so RoPE is 2 tensor muls + a P32 permutation matmul + add.
  - Branch-2 weights block-swapped so the two differential branches use
    complementary 64-partition halves (K=64 score matmuls).
  - Scores transposed S^T[k, q]; exp without max-subtraction; PV with an
    extra ones column giving the softmax denominators.

This version restructures for engine balance + pipelining:
  - ACT does ONLY the exps (plus cheap at-tile copies); the old
    ACT-copy-with-scale normalize step is replaced by a DVE/GPSIMD
    tensor_scalar + scalar_tensor_tensor pair. The lambda fold is baked
    into the PV rhs: vext = [1/lam | v | 1], so branch-2's denominator
    column comes out pre-divided by lambda and one reciprocal yields
    both branch scales. The combined sign flip (pair = lam*a2' - a1') is
    fixed by negating Wout on the host.
  - RoPE: t-mul + final add on DVE, u-mul on GPSIMD, P32 swap on PE.
  - Software pipelining: per batch, head-group mt's attention stages are
    emitted with a 2-stage lag (scores -> exp -> PV+normalize) and
    q-projections for mt+2 are emitted between stages; batch-1
    projections interleave with batch-0's out-projection so the PE never
    idles (stays at 2.4 GHz).
"""

import os
import sys

sys.path.insert(0, "/opt/trn_rl_repo")
os.environ.setdefault("MYCRO_LOCAL_CACHE", "1")

import numpy as np

B, L, E, H, G, W, D = 2, 4096, 1024, 16, 2, 256, 64
NCORES = 8
SEQ = 384          # x rows per core
NW = 2             # windows per core
QROWS = 512        # output rows per core per batch
KV = E // (H // G)  # 128
LAMBDA_INIT = 0.8


# ----------------------------------------------------------------- host prep

def _head_perm():
    """Column permutation applied to Wq1/Wk1: per 64-block -> [evens|odds]."""
    p = []
    for blk in range(0, E, D):
        p += [blk + 2 * j for j in range(D // 2)]
        p += [blk + 2 * j + 1 for j in range(D // 2)]
    return np.array(p, dtype=np.int64)


def _q2_perm():
    """q2: like _head_perm but heads swapped within each 128-col M-tile."""
    base = _head_perm()
    p = np.empty_like(base)
    for m in range(E // 128):
        p[m * 128: m * 128 + 64] = base[m * 128 + 64: m * 128 + 128]
        p[m * 128 + 64: m * 128 + 128] = base[m * 128: m * 128 + 64]
    return p


def _k_perm(swap):
    """kv columns (128 = 2 groups x 64): per group block [evens|odds];
    swap=True puts group1 first (branch-2 layout)."""
    p = []
    groups = (1, 0) if swap else (0, 1)
    for g in groups:
        blk = g * D
        p += [blk + 2 * j for j in range(D // 2)]
        p += [blk + 2 * j + 1 for j in range(D // 2)]
    return np.array(p, dtype=np.int64)


def _tile_w(w, kdim, mdim):
    """(kdim*128, mdim*TS) -> (kdim, mdim, 128, TS) contiguous tiles."""
    ts = w.shape[1] // mdim
    return np.ascontiguousarray(
        w.reshape(kdim, 128, mdim, ts).transpose(0, 2, 1, 3))


def _trig_tables(core):
    pos = (256 * core + np.arange(SEQ, dtype=np.float64))  # global positions
    div = np.exp(np.arange(0, D, 2, dtype=np.float64) * (-np.log(10000.0) / D))
    ang = pos[None, :] * div[:, None]          # (32, SEQ)
    c32 = np.cos(ang).astype(np.float32)
    s32 = np.sin(ang).astype(np.float32)
    tc = np.tile(c32, (4, 1))                   # (128, SEQ)
    # sign-folded sin: rows [0:32]=+sin (qe*sin for the odd half),
    # [32:64]=-sin (-qo*sin for the even half), repeating per 64-block.
    tsn = np.tile(np.concatenate([s32, -s32], axis=0), (2, 1))
    return np.ascontiguousarray(tc), np.ascontiguousarray(tsn)


def _p32():
    """[128,128] permutation: swaps 32-halves within each 64-block.
    Used as matmul lhsT: out = P.T @ u with P[k, m] = 1 iff k = swap(m)."""
    p = np.zeros((128, 128), np.float32)
    for m in range(128):
        k = m + 32 if (m % 64) < 32 else m - 32
        p[k, m] = 1.0
    return p


# ------------------------------------------------------------ device program

_PROGRAM_CACHE = {}


def _build_program():
    import concourse.bass as bass
    import concourse.mybir as mybir
    import concourse.tile as tile
    from concourse.masks import make_identity
    from concourse.tile_rust import add_dep_helper

    def order_group(insts):
        """PE-order a bank-packed accumulation group: first (start=True)
        before everything, last (stop=True) after everything. sync=False —
        same-engine ordering only."""
        first, last = insts[0], insts[-1]
        for i in insts[1:]:
            add_dep_helper(i.ins, first.ins, sync=False,
                           reason="psum group start first")
        for i in insts[:-1]:
            add_dep_helper(last.ins, i.ins, sync=False,
                           reason="psum group stop last")

    f32 = mybir.dt.float32
    f32r = mybir.dt.float32r
    bf16 = mybir.dt.bfloat16
    ALU = mybir.AluOpType
    ACTF = mybir.ActivationFunctionType

    nc = bass.Bass()

    # All inputs partition-major and contiguous per partition row so each
    # DMA is a single max-bandwidth linear transfer.
    xt_d = nc.dram_tensor("xt", [128, B * 8 * SEQ], bf16,
                          kind="ExternalInput")
    # wq pre-chunked on host: [chunk=(mat,mt//2), 128p, (mt%2, kt, 128)]
    wq_d = nc.dram_tensor("wq", [8, 128, 2048], bf16, kind="ExternalInput")
    wk_d = nc.dram_tensor("wk", [128, 2048], bf16, kind="ExternalInput")
    wv_d = nc.dram_tensor("wv", [128, 1024], bf16, kind="ExternalInput")
    wo_d = nc.dram_tensor("wo", [128, 8192], bf16, kind="ExternalInput")
    tc_d = nc.dram_tensor("tct", [128, SEQ], bf16, kind="ExternalInput")
    ts_d = nc.dram_tensor("tst", [128, SEQ], bf16, kind="ExternalInput")
    lam_d = nc.dram_tensor("lamv", [128, 2], f32, kind="ExternalInput")
    bout_d = nc.dram_tensor("boutv", [1, E], f32r, kind="ExternalInput")
    p32_d = nc.dram_tensor("p32", [128, 128], bf16, kind="ExternalInput")
    ones_d = nc.dram_tensor("onesv", [1, 128], f32r, kind="ExternalInput")
    y_d = nc.dram_tensor("y", [B, QROWS, E], bf16, kind="ExternalOutput")

    def split_matmul_waits():
        """This walrus build allows only ONE sync-wait per engine
        instruction; peel extra waits onto engine-matched no-ops placed
        just before the instruction."""
        for bb in nc.m.functions[0].blocks:
            il = bb.instructions
            new_list = []
            changed = False
            for i in il:
                si = getattr(i, "sync_info", None)
                if si is not None and len(si.on_wait) > 1:
                    waits = list(si.on_wait)
                    for j, w in enumerate(waits[1:]):
                        nop = mybir.InstNoOp(
                            name=f"{i.name}-wnop{j}", engine=i.engine, ins=[],
                            outs=[],
                            sync_info=mybir.SyncInfo(on_wait=[w],
                                                     on_update=[]))
                        nc.inst_map[nop.name] = nop
                        new_list.append(nop)
                    i.sync_info = mybir.SyncInfo(
                        on_wait=[waits[0]], on_update=list(si.on_update))
                    changed = True
                new_list.append(i)
            if changed:
                il[:] = new_list

    with tile.TileContext(nc) as tc:
        with tc.tile_pool(name="const", bufs=1) as constp, \
             tc.tile_pool(name="xt", bufs=1) as xtp, \
             tc.tile_pool(name="rot", bufs=1) as rotp, \
             tc.tile_pool(name="wres", bufs=1) as wresp, \
             tc.tile_pool(name="ru", bufs=3) as rup, \
             tc.tile_pool(name="vext", bufs=1) as vxp, \
             tc.tile_pool(name="att", bufs=4) as attp, \
             tc.tile_pool(name="small", bufs=4) as smp, \
             tc.tile_pool(name="pairs", bufs=1) as pairp, \
             tc.tile_pool(name="atile", bufs=1) as atp, \
             tc.tile_pool(name="ysb", bufs=3) as ysbp, \
             tc.tile_pool(name="psSC", bufs=2, space="PSUM") as psSC, \
             tc.tile_pool(name="psPV", bufs=2, space="PSUM") as psPV, \
             tc.tile_pool(name="psA", bufs=2, space="PSUM") as psA:

            # ---- input DMAs: two queues in parallel. sync: batch-0 x +
            # k/v weights + consts (everything the prologue needs, ~1.3MB);
            # gpsimd: wq + batch-1 x + wo (6.8MB, needed later).
            xts = {}
            xstrip = xtp.tile([128, B * 8 * SEQ], bf16, tag="xt",
                              name="xstrip")
            for b in range(B):
                for kt in range(8):
                    off = (b * 8 + kt) * SEQ
                    xts[b, kt] = xstrip[:, off:off + SEQ]

            nc.sync.dma_start(out=xstrip[:, 0:8 * SEQ],
                              in_=xt_d[:, 0:8 * SEQ])
            wkr = wresp.tile([128, 2048], bf16, tag="wkr", name="wkr")
            nc.gpsimd.dma_start(out=wkr, in_=wk_d[:, :])
            wvr = wresp.tile([128, 1024], bf16, tag="wvr", name="wvr")
            nc.gpsimd.dma_start(out=wvr, in_=wv_d[:, :])

            tc_sb = constp.tile([128, SEQ], bf16, tag="tcs", name="tc_sb")
            ts_sb = constp.tile([128, SEQ], bf16, tag="tss", name="ts_sb")
            p32_sb = constp.tile([128, 128], bf16, tag="p32s", name="p32_sb")
            lam_sb = constp.tile([128, 2], f32, tag="lams", name="lam_sb")
            bout_sb = constp.tile([1, E], f32r, tag="bouts", name="bout_sb")
            ones1 = constp.tile([1, 128], f32r, tag="ones1", name="ones1")
            nc.sync.dma_start(out=tc_sb, in_=tc_d[:, :])
            nc.sync.dma_start(out=ts_sb, in_=ts_d[:, :])
            nc.sync.dma_start(out=p32_sb, in_=p32_d[:, :])
            nc.sync.dma_start(out=lam_sb, in_=lam_d[:, :])
            nc.sync.dma_start(out=bout_sb, in_=bout_d[:, :])
            nc.sync.dma_start(out=ones1, in_=ones_d[:, :])

            wqr = wresp.tile([128, 16384], bf16, tag="wqr", name="wqr")

            # chunk = mat*4 + mt//2; order so both mats of each mt-pair
            # arrive together, in mt order.
            for chunk in (0, 4, 1, 5, 2, 6, 3, 7):
                nc.gpsimd.dma_start(
                    out=wqr[:, chunk * 2048:(chunk + 1) * 2048],
                    in_=wq_d[chunk, :, :])
            nc.gpsimd.dma_start(out=xstrip[:, 8 * SEQ:16 * SEQ],
                                in_=xt_d[:, 8 * SEQ:16 * SEQ])
            wor = wresp.tile([128, 8192], bf16, tag="wor", name="wor")
            nc.gpsimd.dma_start(out=wor, in_=wo_d[:, :])
            wo_sb = {}
            for kt in range(8):
                for nh in range(2):
                    off = kt * 1024 + nh * 512
                    wo_sb[kt, nh] = wor[:, off:off + 512]

            def wq_sb(mat, mt, kt):
                off = (mat * 4 + mt // 2) * 2048 + (mt % 2) * 1024 + kt * 128
                return wqr[:, off:off + 128]

            identf = constp.tile([128, 128], f32, tag="identf", name="identf")
            make_identity(nc, identf)
            identb = constp.tile([128, 128], bf16, tag="identb", name="identb")
            nc.vector.tensor_copy(identb, identf)

            # ---- shared state ----
            qrot, krot, vext, pairs, at2s = {}, {}, {}, {}, {}
            bias_sb = constp.tile([128, E], f32, tag="biasbc", name="bias_sb")

            # RoPE is split so its 4-engine chain never head-of-line blocks
            # the PE queue: the cast (ACT, frees the proj psum slot) is
            # emitted right after the projection matmuls; the rot part (GPS
            # muls -> PE P32 -> DVE add) is deferred until other PE work has
            # been queued.
            def rope_cast(psum_in, name):
                qps = rup.tile([128, SEQ], bf16, tag="ropec",
                               name=f"c_{name}")
                nc.scalar.activation(qps, psum_in, ACTF.Copy)
                return qps

            def rope_rot(qps, rot_out, name):
                t = rup.tile([128, SEQ], bf16, tag="ropet", name=f"t_{name}")
                u = rup.tile([128, SEQ], bf16, tag="ropeu", name=f"u_{name}")
                nc.gpsimd.tensor_mul(t, qps, tc_sb)
                nc.gpsimd.tensor_mul(u, qps, ts_sb)
                uswt = psA.tile([128, 512], f32, tag="A", name=f"usw_{name}")
                usw = uswt[:, 0:SEQ]
                nc.tensor.matmul(usw, p32_sb, u, start=True, stop=True)
                nc.vector.tensor_add(rot_out, t, usw)

            def qproj_cast(b, mt):
                casts = []
                for mat in range(2):
                    ps = psA.tile([128, 512], f32, tag="A",
                                  name=f"qp{b}_{mat}_{mt}")
                    qp = ps[:, 0:SEQ]
                    for kt in range(8):
                        nc.tensor.matmul(
                            qp, wq_sb(mat, mt, kt), xts[b, kt],
                            start=(kt == 0), stop=(kt == 7))
                    casts.append(rope_cast(qp, f"q{mat}_{b}_{mt}"))
                return casts

            def qproj_finish(b, mt, casts):
                for mat in range(2):
                    rot = rotp.tile([128, SEQ], bf16, tag=f"q{mat}_{b}_{mt}",
                                    name=f"qr{mat}_{b}_{mt}")
                    rope_rot(casts[mat], rot, f"q{mat}_{b}_{mt}")
                    qrot[mat, b, mt] = rot

            def kv_cast(b):
                st = {}
                for mat in range(2):
                    ps = psA.tile([128, 512], f32, tag="A",
                                  name=f"kp{b}_{mat}")
                    kp = ps[:, 0:SEQ]
                    for kt in range(8):
                        nc.tensor.matmul(
                            kp, wkr[:, mat * 1024 + kt * 128:
                                    mat * 1024 + kt * 128 + 128],
                            xts[b, kt],
                            start=(kt == 0), stop=(kt == 7))
                    st[f"k{mat}"] = rope_cast(kp, f"k{mat}_{b}")
                # v^T at full rate
                ps = psA.tile([128, 512], f32, tag="A", name=f"vp{b}")
                vtp = ps[:, 0:SEQ]
                for kt in range(8):
                    nc.tensor.matmul(
                        vtp, wvr[:, kt * 128:(kt + 1) * 128],
                        xts[b, kt],
                        start=(kt == 0), stop=(kt == 7))
                vt_sb = rup.tile([128, SEQ], bf16, tag="vtsb",
                                 name=f"vt_sb{b}")
                nc.vector.tensor_copy(vt_sb, vtp)
                st["vt"] = vt_sb
                return st

            def kv_finish(b, st):
                for mat in range(2):
                    rot = rotp.tile([128, SEQ], bf16, tag=f"k{mat}_{b}",
                                    name=f"kr{mat}_{b}")
                    rope_rot(st[f"k{mat}"], rot, f"k{mat}_{b}")
                    krot[mat, b] = rot
                vt_sb = st["vt"]
                for s in range(3):
                    vtr = psA.tile([128, 256], bf16, tag="A",
                                   name=f"vtr{b}_{s}")
                    nc.tensor.matmul(vtr[:, 0:128],
                                     vt_sb[:, s * 128:(s + 1) * 128],
                                     identb, is_transpose=True)
                    for g in range(2):
                        # vext layout: [1/lam | v(64) | 1]; branch0 rhs =
                        # cols 1:66 ([v|1]), branch1 rhs = cols 0:65
                        # ([1/lam|v]).
                        ve = vxp.tile([128, 66], bf16, tag=f"ve_{b}_{s}_{g}",
                                      name=f"ve{b}_{s}_{g}")
                        nc.vector.tensor_copy(ve[:, 1:65],
                                              vtr[:, g * 64:(g + 1) * 64])
                        nc.gpsimd.tensor_copy(ve[:, 0:1], lam_sb[:, 0:1])
                        nc.gpsimd.tensor_copy(ve[:, 65:66], lam_sb[:, 1:2])
                        vext[b, s, g] = ve

            def bias_broadcast():
                for nh in range(2):
                    bps = psA.tile([128, 512], f32, tag="A", name=f"bps{nh}")
                    nc.tensor.matmul(bps, ones1,
                                     bout_sb[:, nh * 512:(nh + 1) * 512],
                                     start=True, stop=True)
                    nc.vector.tensor_copy(
                        bias_sb[:, nh * 512:(nh + 1) * 512], bps)

            # ---- attention stages (software-pipelined per batch) ----

            def stage_A(b, i, hw, ctx):
                mt, h, w = hw
                g = h & 1
                base1 = 64 * g
                base2 = 64 - base1
                # both branches' scores in one 2-bank psum tile so a single
                # N=1024 exp covers the whole head-window.
                st_ps = psSC.tile([128, 1024], f32, tag="sc",
                                  name=f"sc{b}_{i}")
                for br, qb in ((0, base1), (1, base2)):
                    mms = []
                    for kts in range(2):
                        mms.append(nc.tensor.matmul(
                            st_ps[:, br * 512 + kts * 256:
                                  br * 512 + kts * 256 + 256],
                            krot[br, b][qb:qb + 64,
                                        w * 128 + kts * 128:
                                        w * 128 + kts * 128 + 128],
                            qrot[br, b, mt][qb:qb + 64,
                                            w * 128:w * 128 + 256],
                            start=(kts == 0), stop=(kts == 1)))
                    order_group(mms)
                ctx[i] = {"sc": st_ps}

            def stage_B(b, i, hw, ctx):
                e = attp.tile([128, 1024], bf16, tag="e",
                              name=f"e{b}_{i}")
                nc.scalar.activation(e, ctx[i]["sc"], ACTF.Exp,
                                     scale=0.125)
                ctx[i]["e"] = e

            def stage_CD(b, i, hw, ctx):
                mt, h, w = hw
                g = h & 1
                e = ctx[i]["e"]
                pv = psPV.tile([128, 260], f32, tag="pv", name=f"pv{b}_{i}")
                mms = []
                first = True
                for kts in range(2):
                    for qt in range(2):
                        for br in range(2):
                            col = qt * 130 + br * 65
                            ve = vext[b, w + kts, g]
                            rhs = ve[:, 0:65] if br == 1 else ve[:, 1:66]
                            mms.append(nc.tensor.matmul(
                                pv[:, col:col + 65],
                                e[:, br * 512 + kts * 256 + qt * 128:
                                  br * 512 + kts * 256 + qt * 128 + 128],
                                rhs,
                                start=first,
                                stop=(kts == 1 and qt == 1 and br == 1)))
                            first = False
                order_group(mms)

                # denominators: cols 64,65 (qt0: r1, lam*r2) and 194,195
                r_sb = smp.tile([128, 4], f32, tag="recip", name=f"r{b}_{i}")
                nc.vector.reciprocal(
                    r_sb.rearrange("p (c k) -> p c k", c=2),
                    pv.rearrange("p (c k) -> p c k", c=2)[:, :, 64:66])
                if g == 0:
                    for qt in range(2):
                        pairs[b, mt, w, qt] = pairp.tile(
                            [128, 128], bf16, tag=f"pair{b}_{mt}_{w}_{qt}",
                            name=f"pair{b}_{mt}_{w}_{qt}")
                for qt in range(2):
                    t1 = smp.tile([128, 64], bf16, tag=f"t1_{qt}",
                                  name=f"t1_{b}_{i}_{qt}")
                    nc.vector.tensor_scalar_mul(
                        t1, pv[:, qt * 130:qt * 130 + 64],
                        r_sb[:, 2 * qt:2 * qt + 1])
                    # pair = (pv_br2 * lam*r2) - pv_br1*r1 = -(a); Wout is
                    # negated on the host to compensate.
                    nc.vector.scalar_tensor_tensor(
                        out=pairs[b, mt, w, qt][:, g * 64:(g + 1) * 64],
                        in0=pv[:, qt * 130 + 66:qt * 130 + 130],
                        scalar=r_sb[:, 2 * qt + 1:2 * qt + 2],
                        in1=t1, op0=ALU.mult, op1=ALU.subtract)

            def stage_E(b, mt, w):
                tr = psA.tile([128, 256], bf16, tag="A", name=f"tr{b}_{mt}_{w}")
                for qt in range(2):
                    nc.tensor.transpose(tr[:, qt * 128:(qt + 1) * 128],
                                        pairs[b, mt, w, qt], identb)
                at = atp.tile([128, 256], bf16, tag=f"at{b}_{w}_{mt}",
                              name=f"at{b}_{w}_{mt}")
                nc.vector.tensor_copy(at, tr)
                at2s[b, w, mt] = at

            def run_pipeline(b, fillers=None):
                hws = [(mt, 2 * mt + gg, w)
                       for mt in range(8) for gg in (0, 1) for w in (0, 1)]
                n = len(hws)
                ctx = {}
                pend = {}
                for i in range(n + 2):
                    if i < n:
                        stage_A(b, i, hws[i], ctx)
                    if 0 <= i - 1 < n:
                        stage_B(b, i - 1, hws[i - 1], ctx)
                    if i < n:
                        # q-projection prefetch, two groups ahead; emitted
                        # after this step's exps so the ACT cast doesn't
                        # delay them.
                        mt, h, w = hws[i]
                        if h == 2 * mt and w == 0:
                            if mt + 1 in pend:
                                qproj_finish(b, mt + 1, pend.pop(mt + 1))
                            if mt + 2 < 8:
                                pend[mt + 2] = qproj_cast(b, mt + 2)
                            if fillers and mt in fillers:
                                for thunk in fillers[mt]:
                                    thunk()
                    if 0 <= i - 2 < n:
                        j = i - 2
                        stage_CD(b, j, hws[j], ctx)
                        mt, h, w = hws[j]
                        if h % 2 == 1:
                            stage_E(b, mt, w)
                        del ctx[j]

            def outproj(b, w):
                for qt in range(2):
                    for nh in range(2):
                        y_ps = psA.tile([128, 512], f32, tag="A",
                                        name=f"y{b}_{w}_{qt}_{nh}")
                        mms = []
                        for kt in range(8):
                            mms.append(nc.tensor.matmul(
                                y_ps,
                                at2s[b, w, kt][:, qt * 128:(qt + 1) * 128],
                                wo_sb[kt, nh],
                                start=(kt == 0), stop=(kt == 7)))
                        order_group(mms)
                        y_sb = ysbp.tile([128, 512], bf16, tag="ysb",
                                         name=f"ysb{b}_{w}_{qt}_{nh}")
                        nc.vector.tensor_add(
                            y_sb, y_ps,
                            bias_sb[:, nh * 512:(nh + 1) * 512])
                        dma_eng = nc.gpsimd if (qt + nh) % 2 == 0 else nc.sync
                        dma_eng.dma_start(
                            out=y_d[b, (w * 2 + qt) * 128:
                                    (w * 2 + qt) * 128 + 128,
                                    nh * 512:(nh + 1) * 512],
                            in_=y_sb)

            # ---- emission schedule ----
            st0 = kv_cast(0)
            c00 = qproj_cast(0, 0)
            c01 = qproj_cast(0, 1)
            bias_broadcast()
            kv_finish(0, st0)
            qproj_finish(0, 0, c00)
            qproj_finish(0, 1, c01)
            # batch-1 projections interleave with batch-0's last pipeline
            # groups so the batch boundary has no ACT/DVE idle stretch.
            bx = {}
            fillers = {
                5: [lambda: bx.__setitem__("st1", kv_cast(1))],
                6: [lambda: bx.__setitem__("c10", qproj_cast(1, 0)),
                    lambda: kv_finish(1, bx["st1"])],
                7: [lambda: bx.__setitem__("c11", qproj_cast(1, 1)),
                    lambda: qproj_finish(1, 0, bx["c10"])],
            }
            run_pipeline(0, fillers)
            qproj_finish(1, 1, bx["c11"])
            outproj(0, 0)
            outproj(0, 1)
            run_pipeline(1)
            outproj(1, 0)
            outproj(1, 1)

    split_matmul_waits()
    return nc


def get_program():
    if "nc" not in _PROGRAM_CACHE:
        _PROGRAM_CACHE["nc"] = _build_program()
    return _PROGRAM_CACHE["nc"]


# ------------------------------------------------------------------ host API

def make_in_maps(x, Wq1, Wq2, Wk1, Wk2, Wv, Wout, bout, lq1, lk1, lq2, lk2):
    import ml_dtypes
    bf16 = ml_dtypes.bfloat16

    x = np.asarray(x, dtype=np.float32)
    lam = float(np.clip(
        np.exp(np.asarray(lq1, np.float64) @ np.asarray(lk1, np.float64))
        - np.exp(np.asarray(lq2, np.float64) @ np.asarray(lk2, np.float64))
        + LAMBDA_INIT, 0.1, 0.9))

    qp1, qp2 = _head_perm(), _q2_perm()
    kp1, kp2 = _k_perm(False), _k_perm(True)

    wq_t = np.stack([
        _tile_w(np.asarray(Wq1, np.float32)[:, qp1], 8, 8),
        _tile_w(np.asarray(Wq2, np.float32)[:, qp2], 8, 8),
    ])  # (mat, kt, mt, 128, 128)
    # chunk layout for single contiguous DMAs:
    # (mat, mt//2, p, mt%2, kt, s) -> (8, 128, 2048)
    wq = np.ascontiguousarray(
        wq_t.reshape(2, 8, 4, 2, 128, 128)
        .transpose(0, 2, 4, 3, 1, 5).reshape(8, 128, 2048)).astype(bf16)
    # wk/wv/wo partition-major contiguous: [128, ...]
    wk = np.ascontiguousarray(np.stack([
        _tile_w(np.asarray(Wk1, np.float32)[:, kp1], 8, 1)[:, 0],
        _tile_w(np.asarray(Wk2, np.float32)[:, kp2], 8, 1)[:, 0],
    ]).transpose(2, 0, 1, 3).reshape(128, 2048)).astype(bf16)
    wv = np.ascontiguousarray(
        _tile_w(np.asarray(Wv, np.float32), 8, 1)[:, 0]
        .transpose(1, 0, 2).reshape(128, 1024)).astype(bf16)
    # NOTE: negated — the device computes -a (branch2-scaled minus branch1).
    wo = np.ascontiguousarray(
        _tile_w(-np.asarray(Wout, np.float32), 8, 2)
        .transpose(2, 0, 1, 3).reshape(128, 8192)).astype(bf16)
    boutv = np.asarray(bout, np.float32).reshape(1, E)

    lamv = np.zeros((128, 2), np.float32)
    lamv[:, 0] = 1.0 / lam  # branch-2 "ones" column -> den2/lam
    lamv[:, 1] = 1.0        # branch-1 ones column

    # x^T, tiled: (B, 8, 128, SEQ) per core
    xT = np.ascontiguousarray(x.transpose(0, 2, 1))  # (B, E, L)

    in_maps = []
    for c in range(NCORES):
        s0 = 256 * c
        xt = np.ascontiguousarray(
            xT[:, :, s0:s0 + SEQ].reshape(B, 8, 128, SEQ)
            .transpose(2, 0, 1, 3).reshape(128, B * 8 * SEQ)).astype(bf16)
        tct, tst = _trig_tables(c)
        in_maps.append({
            "xt": xt, "wq": wq, "wk": wk, "wv": wv, "wo": wo,
            "tct": tct.astype(bf16), "tst": tst.astype(bf16),
            "lamv": lamv, "boutv": boutv,
            "p32": _p32().astype(bf16),
            "onesv": np.ones((1, 128), np.float32),
        })
    return in_maps


def kernel(**inputs) -> np.ndarray:
    from concourse.bass_utils import run_bass_kernel_spmd

    in_maps = make_in_maps(**inputs)
    nc = get_program()
    res = run_bass_kernel_spmd(nc, in_maps, core_ids=list(range(NCORES)))
    out = np.empty((B, L, E), dtype=np.float32)
    for c in range(NCORES):
        out[:, 512 * c:512 * (c + 1), :] = \
            res.results[c]["y"].astype(np.float32)
    return out


# revision 30
# speedup vs baseline: 1.4005x; 1.0110x over previous
"""Trainium2 Bass kernel for EnhancedLocalAttentionWithGQA (differential
windowed attention, B=2 L=4096 E=1024 H=16 G=2 W=256 D=64).

Structural facts exploited (same as prior version):
  - Only windows 0..15 contribute; core c owns windows (2c, 2c+1) ->
    output rows [512c, 512c+512) per batch; needs x rows [256c, 256c+384).
  - q^T/k^T computed in [head-dim, seq] layout with host-permuted weight
    columns so RoPE is 2 tensor muls + a P32 permutation matmul + add.
  - Branch-2 weights block-swapped so the two differential branches use
    complementary 64-partition halves (K=64 score matmuls).
  - Scores transposed S^T[k, q]; exp without max-subtraction; PV with an
    extra ones column giving the softmax denominators.

This version restructures for engine balance + pipelining:
  - ACT does ONLY the exps (plus cheap at-tile copies); the old
    ACT-copy-with-scale normalize step is replaced by a DVE/GPSIMD
    tensor_scalar + scalar_tensor_tensor pair. The lambda fold is baked
    into the PV rhs: vext = [1/lam | v | 1], so branch-2's denominator
    column comes out pre-divided by lambda and one reciprocal yields
    both branch scales. The combined sign flip (pair = lam*a2' - a1') is
    fixed by negating Wout on the host.
  - RoPE: t-mul + final add on DVE, u-mul on GPSIMD, P32 swap on PE.
  - Software pipelining: per batch, head-group mt's attention stages are
    emitted with a 2-stage lag (scores -> exp -> PV+normalize) and
    q-projections for mt+2 are emitted between stages; batch-1
    projections interleave with batch-0's out-projection so the PE never
    idles (stays at 2.4 GHz).
"""

import os
import sys

sys.path.insert(0, "/opt/trn_rl_repo")
os.environ.setdefault("MYCRO_LOCAL_CACHE", "1")

import numpy as np

B, L, E, H, G, W, D = 2, 4096, 1024, 16, 2, 256, 64
NCORES = 8
SEQ = 384          # x rows per core
NW = 2             # windows per core
QROWS = 512        # output rows per core per batch
KV = E // (H // G)  # 128
LAMBDA_INIT = 0.8


# ----------------------------------------------------------------- host prep

def _head_perm():
    """Column permutation applied to Wq1/Wk1: per 64-block -> [evens|odds]."""
    p = []
    for blk in range(0, E, D):
        p += [blk + 2 * j for j in range(D // 2)]
        p += [blk + 2 * j + 1 for j in range(D // 2)]
    return np.array(p, dtype=np.int64)


def _q2_perm():
    """q2: like _head_perm but heads swapped within each 128-col M-tile."""
    base = _head_perm()
    p = np.empty_like(base)
    for m in range(E // 128):
        p[m * 128: m * 128 + 64] = base[m * 128 + 64: m * 128 + 128]
        p[m * 128 + 64: m * 128 + 128] = base[m * 128: m * 128 + 64]
    return p


def _k_perm(swap):
    """kv columns (128 = 2 groups x 64): per group block [evens|odds];
    swap=True puts group1 first (branch-2 layout)."""
    p = []
    groups = (1, 0) if swap else (0, 1)
    for g in groups:
        blk = g * D
        p += [blk + 2 * j for j in range(D // 2)]
        p += [blk + 2 * j + 1 for j in range(D // 2)]
    return np.array(p, dtype=np.int64)


def _tile_w(w, kdim, mdim):
    """(kdim*128, mdim*TS) -> (kdim, mdim, 128, TS) contiguous tiles."""
    ts = w.shape[1] // mdim
    return np.ascontiguousarray(
        w.reshape(kdim, 128, mdim, ts).transpose(0, 2, 1, 3))


def _trig_tables(core):
    pos = (256 * core + np.arange(SEQ, dtype=np.float64))  # global positions
    div = np.exp(np.arange(0, D, 2, dtype=np.float64) * (-np.log(10000.0) / D))
    ang = pos[None, :] * div[:, None]          # (32, SEQ)
    c32 = np.cos(ang).astype(np.float32)
    s32 = np.sin(ang).astype(np.float32)
    tc = np.tile(c32, (4, 1))                   # (128, SEQ)
    # sign-folded sin: rows [0:32]=+sin (qe*sin for the odd half),
    # [32:64]=-sin (-qo*sin for the even half), repeating per 64-block.
    tsn = np.tile(np.concatenate([s32, -s32], axis=0), (2, 1))
    return np.ascontiguousarray(tc), np.ascontiguousarray(tsn)


def _p32():
    """[128,128] permutation: swaps 32-halves within each 64-block.
    Used as matmul lhsT: out = P.T @ u with P[k, m] = 1 iff k = swap(m)."""
    p = np.zeros((128, 128), np.float32)
    for m in range(128):
        k = m + 32 if (m % 64) < 32 else m - 32
        p[k, m] = 1.0
    return p


# ------------------------------------------------------------ device program

_PROGRAM_CACHE = {}


def _build_program():
    import concourse.bass as bass
    import concourse.mybir as mybir
    import concourse.tile as tile
    from concourse.masks import make_identity
    from concourse.tile_rust import add_dep_helper

    def order_group(insts):
        """PE-order a bank-packed accumulation group: first (start=True)
        before everything, last (stop=True) after everything. sync=False —
        same-engine ordering only."""
        first, last = insts[0], insts[-1]
        for i in insts[1:]:
            add_dep_helper(i.ins, first.ins, sync=False,
                           reason="psum group start first")
        for i in insts[:-1]:
            add_dep_helper(last.ins, i.ins, sync=False,
                           reason="psum group stop last")

    f32 = mybir.dt.float32
    f32r = mybir.dt.float32r
    bf16 = mybir.dt.bfloat16
    ALU = mybir.AluOpType
    ACTF = mybir.ActivationFunctionType

    nc = bass.Bass()

    # All inputs partition-major and contiguous per partition row so each
    # DMA is a single max-bandwidth linear transfer.
    xt_d = nc.dram_tensor("xt", [128, B * 8 * SEQ], bf16,
                          kind="ExternalInput")
    # wq pre-chunked on host: [chunk=(mat,mt//2), 128p, (mt%2, kt, 128)]
    wq_d = nc.dram_tensor("wq", [8, 128, 2048], bf16, kind="ExternalInput")
    wk_d = nc.dram_tensor("wk", [128, 2048], bf16, kind="ExternalInput")
    wv_d = nc.dram_tensor("wv", [128, 1024], bf16, kind="ExternalInput")
    wo_d = nc.dram_tensor("wo", [128, 8192], bf16, kind="ExternalInput")
    tc_d = nc.dram_tensor("tct", [128, SEQ], bf16, kind="ExternalInput")
    ts_d = nc.dram_tensor("tst", [128, SEQ], bf16, kind="ExternalInput")
    lam_d = nc.dram_tensor("lamv", [128, 2], f32, kind="ExternalInput")
    bout_d = nc.dram_tensor("boutv", [1, E], f32r, kind="ExternalInput")
    p32_d = nc.dram_tensor("p32", [128, 128], bf16, kind="ExternalInput")
    ones_d = nc.dram_tensor("onesv", [1, 128], f32r, kind="ExternalInput")
    y_d = nc.dram_tensor("y", [B, QROWS, E], bf16, kind="ExternalOutput")

    def split_matmul_waits():
        """This walrus build allows only ONE sync-wait per engine
        instruction; peel extra waits onto engine-matched no-ops placed
        just before the instruction."""
        for bb in nc.m.functions[0].blocks:
            il = bb.instructions
            new_list = []
            changed = False
            for i in il:
                si = getattr(i, "sync_info", None)
                if si is not None and len(si.on_wait) > 1:
                    waits = list(si.on_wait)
                    for j, w in enumerate(waits[1:]):
                        nop = mybir.InstNoOp(
                            name=f"{i.name}-wnop{j}", engine=i.engine, ins=[],
                            outs=[],
                            sync_info=mybir.SyncInfo(on_wait=[w],
                                                     on_update=[]))
                        nc.inst_map[nop.name] = nop
                        new_list.append(nop)
                    i.sync_info = mybir.SyncInfo(
                        on_wait=[waits[0]], on_update=list(si.on_update))
                    changed = True
                new_list.append(i)
            if changed:
                il[:] = new_list

    with tile.TileContext(nc) as tc:
        with tc.tile_pool(name="const", bufs=1) as constp, \
             tc.tile_pool(name="xt", bufs=1) as xtp, \
             tc.tile_pool(name="rot", bufs=1) as rotp, \
             tc.tile_pool(name="wres", bufs=1) as wresp, \
             tc.tile_pool(name="ru", bufs=3) as rup, \
             tc.tile_pool(name="vext", bufs=1) as vxp, \
             tc.tile_pool(name="att", bufs=4) as attp, \
             tc.tile_pool(name="small", bufs=4) as smp, \
             tc.tile_pool(name="pairs", bufs=1) as pairp, \
             tc.tile_pool(name="atile", bufs=1) as atp, \
             tc.tile_pool(name="ysb", bufs=3) as ysbp, \
             tc.tile_pool(name="psSC", bufs=2, space="PSUM") as psSC, \
             tc.tile_pool(name="psPV", bufs=2, space="PSUM") as psPV, \
             tc.tile_pool(name="psA", bufs=2, space="PSUM") as psA:

            # ---- input DMAs: two queues in parallel. sync: batch-0 x +
            # k/v weights + consts (everything the prologue needs, ~1.3MB);
            # gpsimd: wq + batch-1 x + wo (6.8MB, needed later).
            xts = {}
            xstrip = xtp.tile([128, B * 8 * SEQ], bf16, tag="xt",
                              name="xstrip")
            for b in range(B):
                for kt in range(8):
                    off = (b * 8 + kt) * SEQ
                    xts[b, kt] = xstrip[:, off:off + SEQ]

            # Two DMA queues fed in parallel; tiny consts first on sync so
            # the bias-broadcast matmuls warm the PE immediately; batch-0 x
            # split across both queues in kt order so the k-projection can
            # start as soon as the first chunks + wk land.
            bout_sb = constp.tile([1, E], f32r, tag="bouts", name="bout_sb")
            ones1 = constp.tile([1, 128], f32r, tag="ones1", name="ones1")
            nc.sync.dma_start(out=bout_sb, in_=bout_d[:, :])
            nc.sync.dma_start(out=ones1, in_=ones_d[:, :])
            wkr = wresp.tile([128, 2048], bf16, tag="wkr", name="wkr")
            nc.sync.dma_start(out=wkr, in_=wk_d[:, :])
            nc.sync.dma_start(out=xstrip[:, 0:4 * SEQ],
                              in_=xt_d[:, 0:4 * SEQ])
            nc.gpsimd.dma_start(out=xstrip[:, 4 * SEQ:8 * SEQ],
                                in_=xt_d[:, 4 * SEQ:8 * SEQ])
            wvr = wresp.tile([128, 1024], bf16, tag="wvr", name="wvr")
            nc.gpsimd.dma_start(out=wvr, in_=wv_d[:, :])

            tc_sb = constp.tile([128, SEQ], bf16, tag="tcs", name="tc_sb")
            ts_sb = constp.tile([128, SEQ], bf16, tag="tss", name="ts_sb")
            p32_sb = constp.tile([128, 128], bf16, tag="p32s", name="p32_sb")
            lam_sb = constp.tile([128, 2], f32, tag="lams", name="lam_sb")
            nc.gpsimd.dma_start(out=tc_sb, in_=tc_d[:, :])
            nc.gpsimd.dma_start(out=ts_sb, in_=ts_d[:, :])
            nc.gpsimd.dma_start(out=p32_sb, in_=p32_d[:, :])
            nc.gpsimd.dma_start(out=lam_sb, in_=lam_d[:, :])

            wqr = wresp.tile([128, 16384], bf16, tag="wqr", name="wqr")

            # chunk = mat*4 + mt//2; order so both mats of each mt-pair
            # arrive together, in mt order; split across both queues.
            for qi, chunk in enumerate((0, 4, 1, 5, 2, 6, 3, 7)):
                eng = nc.sync if qi % 2 == 0 else nc.gpsimd
                eng.dma_start(
                    out=wqr[:, chunk * 2048:(chunk + 1) * 2048],
                    in_=wq_d[chunk, :, :])
            nc.sync.dma_start(out=xstrip[:, 8 * SEQ:12 * SEQ],
                              in_=xt_d[:, 8 * SEQ:12 * SEQ])
            nc.gpsimd.dma_start(out=xstrip[:, 12 * SEQ:16 * SEQ],
                                in_=xt_d[:, 12 * SEQ:16 * SEQ])
            wor = wresp.tile([128, 8192], bf16, tag="wor", name="wor")
            nc.sync.dma_start(out=wor[:, 0:4096], in_=wo_d[:, 0:4096])
            nc.gpsimd.dma_start(out=wor[:, 4096:8192], in_=wo_d[:, 4096:8192])
            wo_sb = {}
            for kt in range(8):
                for nh in range(2):
                    off = kt * 1024 + nh * 512
                    wo_sb[kt, nh] = wor[:, off:off + 512]

            def wq_sb(mat, mt, kt):
                off = (mat * 4 + mt // 2) * 2048 + (mt % 2) * 1024 + kt * 128
                return wqr[:, off:off + 128]

            identf = constp.tile([128, 128], f32, tag="identf", name="identf")
            make_identity(nc, identf)
            identb = constp.tile([128, 128], bf16, tag="identb", name="identb")
            nc.vector.tensor_copy(identb, identf)

            # ---- shared state ----
            qrot, krot, vext, pairs, at2s = {}, {}, {}, {}, {}
            bias_sb = constp.tile([128, E], f32, tag="biasbc", name="bias_sb")

            # RoPE is split so its 4-engine chain never head-of-line blocks
            # the PE queue: the cast (ACT, frees the proj psum slot) is
            # emitted right after the projection matmuls; the rot part (GPS
            # muls -> PE P32 -> DVE add) is deferred until other PE work has
            # been queued.
            def rope_cast(psum_in, name):
                qps = rup.tile([128, SEQ], bf16, tag="ropec",
                               name=f"c_{name}")
                nc.scalar.activation(qps, psum_in, ACTF.Copy)
                return qps

            def rope_rot(qps, rot_out, name):
                t = rup.tile([128, SEQ], bf16, tag="ropet", name=f"t_{name}")
                u = rup.tile([128, SEQ], bf16, tag="ropeu", name=f"u_{name}")
                nc.gpsimd.tensor_mul(t, qps, tc_sb)
                nc.gpsimd.tensor_mul(u, qps, ts_sb)
                uswt = psA.tile([128, 512], f32, tag="A", name=f"usw_{name}")
                usw = uswt[:, 0:SEQ]
                nc.tensor.matmul(usw, p32_sb, u, start=True, stop=True)
                nc.vector.tensor_add(rot_out, t, usw)

            def qproj_cast(b, mt):
                casts = []
                for mat in range(2):
                    ps = psA.tile([128, 512], f32, tag="A",
                                  name=f"qp{b}_{mat}_{mt}")
                    qp = ps[:, 0:SEQ]
                    for kt in range(8):
                        nc.tensor.matmul(
                            qp, wq_sb(mat, mt, kt), xts[b, kt],
                            start=(kt == 0), stop=(kt == 7))
                    casts.append(rope_cast(qp, f"q{mat}_{b}_{mt}"))
                return casts

            def qproj_finish(b, mt, casts):
                for mat in range(2):
                    rot = rotp.tile([128, SEQ], bf16, tag=f"q{mat}_{b}_{mt}",
                                    name=f"qr{mat}_{b}_{mt}")
                    rope_rot(casts[mat], rot, f"q{mat}_{b}_{mt}")
                    qrot[mat, b, mt] = rot

            def kv_cast(b):
                st = {}
                for mat in range(2):
                    ps = psA.tile([128, 512], f32, tag="A",
                                  name=f"kp{b}_{mat}")
                    kp = ps[:, 0:SEQ]
                    for kt in range(8):
                        nc.tensor.matmul(
                            kp, wkr[:, mat * 1024 + kt * 128:
                                    mat * 1024 + kt * 128 + 128],
                            xts[b, kt],
                            start=(kt == 0), stop=(kt == 7))
                    st[f"k{mat}"] = rope_cast(kp, f"k{mat}_{b}")
                # v^T at full rate
                ps = psA.tile([128, 512], f32, tag="A", name=f"vp{b}")
                vtp = ps[:, 0:SEQ]
                for kt in range(8):
                    nc.tensor.matmul(
                        vtp, wvr[:, kt * 128:(kt + 1) * 128],
                        xts[b, kt],
                        start=(kt == 0), stop=(kt == 7))
                vt_sb = rup.tile([128, SEQ], bf16, tag="vtsb",
                                 name=f"vt_sb{b}")
                nc.vector.tensor_copy(vt_sb, vtp)
                st["vt"] = vt_sb
                return st

            def kv_finish(b, st):
                for mat in range(2):
                    rot = rotp.tile([128, SEQ], bf16, tag=f"k{mat}_{b}",
                                    name=f"kr{mat}_{b}")
                    rope_rot(st[f"k{mat}"], rot, f"k{mat}_{b}")
                    krot[mat, b] = rot
                vt_sb = st["vt"]
                for s in range(3):
                    vtr = psA.tile([128, 256], bf16, tag="A",
                                   name=f"vtr{b}_{s}")
                    nc.tensor.matmul(vtr[:, 0:128],
                                     vt_sb[:, s * 128:(s + 1) * 128],
                                     identb, is_transpose=True)
                    for g in range(2):
                        # vext layout: [1/lam | v(64) | 1]; branch0 rhs =
                        # cols 1:66 ([v|1]), branch1 rhs = cols 0:65
                        # ([1/lam|v]).
                        ve = vxp.tile([128, 66], bf16, tag=f"ve_{b}_{s}_{g}",
                                      name=f"ve{b}_{s}_{g}")
                        nc.vector.tensor_copy(ve[:, 1:65],
                                              vtr[:, g * 64:(g + 1) * 64])
                        nc.gpsimd.tensor_copy(ve[:, 0:1], lam_sb[:, 0:1])
                        nc.gpsimd.tensor_copy(ve[:, 65:66], lam_sb[:, 1:2])
                        vext[b, s, g] = ve

            def bias_broadcast():
                for nh in range(2):
                    bps = psA.tile([128, 512], f32, tag="A", name=f"bps{nh}")
                    nc.tensor.matmul(bps, ones1,
                                     bout_sb[:, nh * 512:(nh + 1) * 512],
                                     start=True, stop=True)
                    nc.vector.tensor_copy(
                        bias_sb[:, nh * 512:(nh + 1) * 512], bps)

            # ---- attention stages (software-pipelined per batch) ----

            def stage_A(b, i, hw, ctx):
                mt, h, w = hw
                g = h & 1
                base1 = 64 * g
                base2 = 64 - base1
                # both branches' scores in one 2-bank psum tile so a single
                # N=1024 exp covers the whole head-window.
                st_ps = psSC.tile([128, 1024], f32, tag="sc",
                                  name=f"sc{b}_{i}")
                for br, qb in ((0, base1), (1, base2)):
                    mms = []
                    for kts in range(2):
                        mms.append(nc.tensor.matmul(
                            st_ps[:, br * 512 + kts * 256:
                                  br * 512 + kts * 256 + 256],
                            krot[br, b][qb:qb + 64,
                                        w * 128 + kts * 128:
                                        w * 128 + kts * 128 + 128],
                            qrot[br, b, mt][qb:qb + 64,
                                            w * 128:w * 128 + 256],
                            start=(kts == 0), stop=(kts == 1)))
                    order_group(mms)
                ctx[i] = {"sc": st_ps}

            def stage_B(b, i, hw, ctx):
                e = attp.tile([128, 1024], bf16, tag="e",
                              name=f"e{b}_{i}")
                nc.scalar.activation(e, ctx[i]["sc"], ACTF.Exp,
                                     scale=0.125)
                ctx[i]["e"] = e

            def stage_CD(b, i, hw, ctx):
                mt, h, w = hw
                g = h & 1
                e = ctx[i]["e"]
                pv = psPV.tile([128, 260], f32, tag="pv", name=f"pv{b}_{i}")
                mms = []
                first = True
                for kts in range(2):
                    for qt in range(2):
                        for br in range(2):
                            col = qt * 130 + br * 65
                            ve = vext[b, w + kts, g]
                            rhs = ve[:, 0:65] if br == 1 else ve[:, 1:66]
                            mms.append(nc.tensor.matmul(
                                pv[:, col:col + 65],
                                e[:, br * 512 + kts * 256 + qt * 128:
                                  br * 512 + kts * 256 + qt * 128 + 128],
                                rhs,
                                start=first,
                                stop=(kts == 1 and qt == 1 and br == 1)))
                            first = False
                order_group(mms)

                # denominators: cols 64,65 (qt0: r1, lam*r2) and 194,195
                r_sb = smp.tile([128, 4], f32, tag="recip", name=f"r{b}_{i}")
                nc.vector.reciprocal(
                    r_sb.rearrange("p (c k) -> p c k", c=2),
                    pv.rearrange("p (c k) -> p c k", c=2)[:, :, 64:66])
                if g == 0:
                    for qt in range(2):
                        pairs[b, mt, w, qt] = pairp.tile(
                            [128, 128], bf16, tag=f"pair{b}_{mt}_{w}_{qt}",
                            name=f"pair{b}_{mt}_{w}_{qt}")
                for qt in range(2):
                    t1 = smp.tile([128, 64], bf16, tag=f"t1_{qt}",
                                  name=f"t1_{b}_{i}_{qt}")
                    nc.vector.tensor_scalar_mul(
                        t1, pv[:, qt * 130:qt * 130 + 64],
                        r_sb[:, 2 * qt:2 * qt + 1])
                    # pair = (pv_br2 * lam*r2) - pv_br1*r1 = -(a); Wout is
                    # negated on the host to compensate.
                    nc.vector.scalar_tensor_tensor(
                        out=pairs[b, mt, w, qt][:, g * 64:(g + 1) * 64],
                        in0=pv[:, qt * 130 + 66:qt * 130 + 130],
                        scalar=r_sb[:, 2 * qt + 1:2 * qt + 2],
                        in1=t1, op0=ALU.mult, op1=ALU.subtract)

            def stage_E(b, mt, w):
                tr = psA.tile([128, 256], bf16, tag="A", name=f"tr{b}_{mt}_{w}")
                for qt in range(2):
                    nc.tensor.transpose(tr[:, qt * 128:(qt + 1) * 128],
                                        pairs[b, mt, w, qt], identb)
                at = atp.tile([128, 256], bf16, tag=f"at{b}_{w}_{mt}",
                              name=f"at{b}_{w}_{mt}")
                nc.vector.tensor_copy(at, tr)
                at2s[b, w, mt] = at

            def run_pipeline(b, fillers=None, post_e=None):
                hws = [(mt, 2 * mt + gg, w)
                       for mt in range(8) for gg in (0, 1) for w in (0, 1)]
                n = len(hws)
                ctx = {}
                pend = {}
                for i in range(n + 2):
                    if i < n:
                        stage_A(b, i, hws[i], ctx)
                    if 0 <= i - 1 < n:
                        stage_B(b, i - 1, hws[i - 1], ctx)
                    if i < n:
                        # q-projection prefetch, two groups ahead; emitted
                        # after this step's exps so the ACT cast doesn't
                        # delay them.
                        mt, h, w = hws[i]
                        if h == 2 * mt and w == 0:
                            if mt + 1 in pend:
                                qproj_finish(b, mt + 1, pend.pop(mt + 1))
                            if mt + 2 < 8:
                                pend[mt + 2] = qproj_cast(b, mt + 2)
                            if fillers and mt in fillers:
                                for thunk in fillers[mt]:
                                    thunk()
                    if 0 <= i - 2 < n:
                        j = i - 2
                        stage_CD(b, j, hws[j], ctx)
                        mt, h, w = hws[j]
                        if h % 2 == 1:
                            stage_E(b, mt, w)
                            if post_e and (mt, w) in post_e:
                                post_e[mt, w]()
                        del ctx[j]

            def outproj(b, w):
                for qt in range(2):
                    for nh in range(2):
                        y_ps = psA.tile([128, 512], f32, tag="A",
                                        name=f"y{b}_{w}_{qt}_{nh}")
                        mms = []
                        for kt in range(8):
                            mms.append(nc.tensor.matmul(
                                y_ps,
                                at2s[b, w, kt][:, qt * 128:(qt + 1) * 128],
                                wo_sb[kt, nh],
                                start=(kt == 0), stop=(kt == 7)))
                        order_group(mms)
                        y_sb = ysbp.tile([128, 512], bf16, tag="ysb",
                                         name=f"ysb{b}_{w}_{qt}_{nh}")
                        nc.vector.tensor_add(
                            y_sb, y_ps,
                            bias_sb[:, nh * 512:(nh + 1) * 512])
                        dma_eng = nc.gpsimd if (qt + nh) % 2 == 0 else nc.sync
                        dma_eng.dma_start(
                            out=y_d[b, (w * 2 + qt) * 128:
                                    (w * 2 + qt) * 128 + 128,
                                    nh * 512:(nh + 1) * 512],
                            in_=y_sb)

            # ---- emission schedule ----
            bias_broadcast()
            st0 = kv_cast(0)
            c00 = qproj_cast(0, 0)
            c01 = qproj_cast(0, 1)
            kv_finish(0, st0)
            qproj_finish(0, 0, c00)
            qproj_finish(0, 1, c01)
            # batch-1 projections interleave with batch-0's last pipeline
            # groups so the batch boundary has no ACT/DVE idle stretch.
            bx = {}
            fillers = {
                5: [lambda: bx.__setitem__("st1", kv_cast(1))],
                6: [lambda: bx.__setitem__("c10", qproj_cast(1, 0)),
                    lambda: kv_finish(1, bx["st1"])],
                7: [lambda: bx.__setitem__("c11", qproj_cast(1, 1)),
                    lambda: qproj_finish(1, 0, bx["c10"])],
            }
            run_pipeline(0, fillers)
            qproj_finish(1, 1, bx["c11"])
            # batch-0 out-projections interleave into batch-1's first
            # pipeline groups; batch-1 w0's starts inside the pipeline as
            # soon as its last at-tile is ready.
            run_pipeline(1,
                         fillers={0: [lambda: outproj(0, 0)],
                                  1: [lambda: outproj(0, 1)]},
                         post_e={(7, 0): lambda: outproj(1, 0)})
            outproj(1, 1)

    split_matmul_waits()
    return nc


def get_program():
    if "nc" not in _PROGRAM_CACHE:
        _PROGRAM_CACHE["nc"] = _build_program()
    return _PROGRAM_CACHE["nc"]


# ------------------------------------------------------------------ host API

def make_in_maps(x, Wq1, Wq2, Wk1, Wk2, Wv, Wout, bout, lq1, lk1, lq2, lk2):
    import ml_dtypes
    bf16 = ml_dtypes.bfloat16

    x = np.asarray(x, dtype=np.float32)
    lam = float(np.clip(
        np.exp(np.asarray(lq1, np.float64) @ np.asarray(lk1, np.float64))
        - np.exp(np.asarray(lq2, np.float64) @ np.asarray(lk2, np.float64))
        + LAMBDA_INIT, 0.1, 0.9))

    qp1, qp2 = _head_perm(), _q2_perm()
    kp1, kp2 = _k_perm(False), _k_perm(True)

    wq_t = np.stack([
        _tile_w(np.asarray(Wq1, np.float32)[:, qp1], 8, 8),
        _tile_w(np.asarray(Wq2, np.float32)[:, qp2], 8, 8),
    ])  # (mat, kt, mt, 128, 128)
    # chunk layout for single contiguous DMAs:
    # (mat, mt//2, p, mt%2, kt, s) -> (8, 128, 2048)
    wq = np.ascontiguousarray(
        wq_t.reshape(2, 8, 4, 2, 128, 128)
        .transpose(0, 2, 4, 3, 1, 5).reshape(8, 128, 2048)).astype(bf16)
    # wk/wv/wo partition-major contiguous: [128, ...]
    wk = np.ascontiguousarray(np.stack([
        _tile_w(np.asarray(Wk1, np.float32)[:, kp1], 8, 1)[:, 0],
        _tile_w(np.asarray(Wk2, np.float32)[:, kp2], 8, 1)[:, 0],
    ]).transpose(2, 0, 1, 3).reshape(128, 2048)).astype(bf16)
    wv = np.ascontiguousarray(
        _tile_w(np.asarray(Wv, np.float32), 8, 1)[:, 0]
        .transpose(1, 0, 2).reshape(128, 1024)).astype(bf16)
    # NOTE: negated — the device computes -a (branch2-scaled minus branch1).
    wo = np.ascontiguousarray(
        _tile_w(-np.asarray(Wout, np.float32), 8, 2)
        .transpose(2, 0, 1, 3).reshape(128, 8192)).astype(bf16)
    boutv = np.asarray(bout, np.float32).reshape(1, E)

    lamv = np.zeros((128, 2), np.float32)
    lamv[:, 0] = 1.0 / lam  # branch-2 "ones" column -> den2/lam
    lamv[:, 1] = 1.0        # branch-1 ones column

    # x^T, tiled: (B, 8, 128, SEQ) per core
    xT = np.ascontiguousarray(x.transpose(0, 2, 1))  # (B, E, L)

    in_maps = []
    for c in range(NCORES):
        s0 = 256 * c
        xt = np.ascontiguousarray(
            xT[:, :, s0:s0 + SEQ].reshape(B, 8, 128, SEQ)
            .transpose(2, 0, 1, 3).reshape(128, B * 8 * SEQ)).astype(bf16)
        tct, tst = _trig_tables(c)
        in_maps.append({
            "xt": xt, "wq": wq, "wk": wk, "wv": wv, "wo": wo,
            "tct": tct.astype(bf16), "tst": tst.astype(bf16),
            "lamv": lamv, "boutv": boutv,
            "p32": _p32().astype(bf16),
            "onesv": np.ones((1, 128), np.float32),
        })
    return in_maps


def kernel(**inputs) -> np.ndarray:
    from concourse.bass_utils import run_bass_kernel_spmd

    in_maps = make_in_maps(**inputs)
    nc = get_program()
    res = run_bass_kernel_spmd(nc, in_maps, core_ids=list(range(NCORES)))
    out = np.empty((B, L, E), dtype=np.float32)
    for c in range(NCORES):
        out[:, 512 * c:512 * (c + 1), :] = \
            res.results[c]["y"].astype(np.float32)
    return out


# revision 33
# speedup vs baseline: 1.4014x; 1.0006x over previous
"""Trainium2 Bass kernel for EnhancedLocalAttentionWithGQA (differential
windowed attention, B=2 L=4096 E=1024 H=16 G=2 W=256 D=64).

Structural facts exploited (same as prior version):
  - Only windows 0..15 contribute; core c owns windows (2c, 2c+1) ->
    output rows [512c, 512c+512) per batch; needs x rows [256c, 256c+384).
  - q^T/k^T computed in [head-dim, seq] layout with host-permuted weight
    columns so RoPE is 2 tensor muls + a P32 permutation matmul + add.
  - Branch-2 weights block-swapped so the two differential branches use
    complementary 64-partition halves (K=64 score matmuls).
  - Scores transposed S^T[k, q]; exp without max-subtraction; PV with an
    extra ones column giving the softmax denominators.

This version restructures for engine balance + pipelining:
  - ACT does ONLY the exps (plus cheap at-tile copies); the old
    ACT-copy-with-scale normalize step is replaced by a DVE/GPSIMD
    tensor_scalar + scalar_tensor_tensor pair. The lambda fold is baked
    into the PV rhs: vext = [1/lam | v | 1], so branch-2's denominator
    column comes out pre-divided by lambda and one reciprocal yields
    both branch scales. The combined sign flip (pair = lam*a2' - a1') is
    fixed by negating Wout on the host.
  - RoPE: t-mul + final add on DVE, u-mul on GPSIMD, P32 swap on PE.
  - Software pipelining: per batch, head-group mt's attention stages are
    emitted with a 2-stage lag (scores -> exp -> PV+normalize) and
    q-projections for mt+2 are emitted between stages; batch-1
    projections interleave with batch-0's out-projection so the PE never
    idles (stays at 2.4 GHz).
"""

import os
import sys

sys.path.insert(0, "/opt/trn_rl_repo")
os.environ.setdefault("MYCRO_LOCAL_CACHE", "1")

import numpy as np

B, L, E, H, G, W, D = 2, 4096, 1024, 16, 2, 256, 64
NCORES = 8
SEQ = 384          # x rows per core
NW = 2             # windows per core
QROWS = 512        # output rows per core per batch
KV = E // (H // G)  # 128
LAMBDA_INIT = 0.8


# ----------------------------------------------------------------- host prep

def _head_perm():
    """Column permutation applied to Wq1/Wk1: per 64-block -> [evens|odds]."""
    p = []
    for blk in range(0, E, D):
        p += [blk + 2 * j for j in range(D // 2)]
        p += [blk + 2 * j + 1 for j in range(D // 2)]
    return np.array(p, dtype=np.int64)


def _q2_perm():
    """q2: like _head_perm but heads swapped within each 128-col M-tile."""
    base = _head_perm()
    p = np.empty_like(base)
    for m in range(E // 128):
        p[m * 128: m * 128 + 64] = base[m * 128 + 64: m * 128 + 128]
        p[m * 128 + 64: m * 128 + 128] = base[m * 128: m * 128 + 64]
    return p


def _k_perm(swap):
    """kv columns (128 = 2 groups x 64): per group block [evens|odds];
    swap=True puts group1 first (branch-2 layout)."""
    p = []
    groups = (1, 0) if swap else (0, 1)
    for g in groups:
        blk = g * D
        p += [blk + 2 * j for j in range(D // 2)]
        p += [blk + 2 * j + 1 for j in range(D // 2)]
    return np.array(p, dtype=np.int64)


def _tile_w(w, kdim, mdim):
    """(kdim*128, mdim*TS) -> (kdim, mdim, 128, TS) contiguous tiles."""
    ts = w.shape[1] // mdim
    return np.ascontiguousarray(
        w.reshape(kdim, 128, mdim, ts).transpose(0, 2, 1, 3))


def _trig_tables(core):
    pos = (256 * core + np.arange(SEQ, dtype=np.float64))  # global positions
    div = np.exp(np.arange(0, D, 2, dtype=np.float64) * (-np.log(10000.0) / D))
    ang = pos[None, :] * div[:, None]          # (32, SEQ)
    c32 = np.cos(ang).astype(np.float32)
    s32 = np.sin(ang).astype(np.float32)
    tc = np.tile(c32, (4, 1))                   # (128, SEQ)
    # sign-folded sin: rows [0:32]=+sin (qe*sin for the odd half),
    # [32:64]=-sin (-qo*sin for the even half), repeating per 64-block.
    tsn = np.tile(np.concatenate([s32, -s32], axis=0), (2, 1))
    return np.ascontiguousarray(tc), np.ascontiguousarray(tsn)


def _p32():
    """[128,128] permutation: swaps 32-halves within each 64-block.
    Used as matmul lhsT: out = P.T @ u with P[k, m] = 1 iff k = swap(m)."""
    p = np.zeros((128, 128), np.float32)
    for m in range(128):
        k = m + 32 if (m % 64) < 32 else m - 32
        p[k, m] = 1.0
    return p


# ------------------------------------------------------------ device program

_PROGRAM_CACHE = {}


def _build_program():
    import concourse.bass as bass
    import concourse.mybir as mybir
    import concourse.tile as tile
    from concourse.masks import make_identity
    from concourse.tile_rust import add_dep_helper

    def order_group(insts):
        """PE-order a bank-packed accumulation group: first (start=True)
        before everything, last (stop=True) after everything. sync=False —
        same-engine ordering only."""
        first, last = insts[0], insts[-1]
        for i in insts[1:]:
            add_dep_helper(i.ins, first.ins, sync=False,
                           reason="psum group start first")
        for i in insts[:-1]:
            add_dep_helper(last.ins, i.ins, sync=False,
                           reason="psum group stop last")

    f32 = mybir.dt.float32
    f32r = mybir.dt.float32r
    bf16 = mybir.dt.bfloat16
    ALU = mybir.AluOpType
    ACTF = mybir.ActivationFunctionType

    nc = bass.Bass()

    # All inputs partition-major and contiguous per partition row so each
    # DMA is a single max-bandwidth linear transfer.
    xt_d = nc.dram_tensor("xt", [128, B * 8 * SEQ], bf16,
                          kind="ExternalInput")
    # wq pre-chunked on host: [chunk=(mat,mt//2), 128p, (mt%2, kt, 128)]
    wq_d = nc.dram_tensor("wq", [8, 128, 2048], bf16, kind="ExternalInput")
    wk_d = nc.dram_tensor("wk", [128, 2048], bf16, kind="ExternalInput")
    wv_d = nc.dram_tensor("wv", [128, 1024], bf16, kind="ExternalInput")
    wo_d = nc.dram_tensor("wo", [128, 8192], bf16, kind="ExternalInput")
    tc_d = nc.dram_tensor("tct", [128, SEQ], bf16, kind="ExternalInput")
    ts_d = nc.dram_tensor("tst", [128, SEQ], bf16, kind="ExternalInput")
    lam_d = nc.dram_tensor("lamv", [128, 2], f32, kind="ExternalInput")
    bout_d = nc.dram_tensor("boutv", [1, E], f32r, kind="ExternalInput")
    p32_d = nc.dram_tensor("p32", [128, 128], bf16, kind="ExternalInput")
    ones_d = nc.dram_tensor("onesv", [1, 128], f32r, kind="ExternalInput")
    y_d = nc.dram_tensor("y", [B, QROWS, E], bf16, kind="ExternalOutput")

    def split_matmul_waits():
        """This walrus build allows only ONE sync-wait per engine
        instruction; peel extra waits onto engine-matched no-ops placed
        just before the instruction."""
        for bb in nc.m.functions[0].blocks:
            il = bb.instructions
            new_list = []
            changed = False
            for i in il:
                si = getattr(i, "sync_info", None)
                if si is not None and len(si.on_wait) > 1:
                    waits = list(si.on_wait)
                    for j, w in enumerate(waits[1:]):
                        nop = mybir.InstNoOp(
                            name=f"{i.name}-wnop{j}", engine=i.engine, ins=[],
                            outs=[],
                            sync_info=mybir.SyncInfo(on_wait=[w],
                                                     on_update=[]))
                        nc.inst_map[nop.name] = nop
                        new_list.append(nop)
                    i.sync_info = mybir.SyncInfo(
                        on_wait=[waits[0]], on_update=list(si.on_update))
                    changed = True
                new_list.append(i)
            if changed:
                il[:] = new_list

    with tile.TileContext(nc) as tc:
        with tc.tile_pool(name="const", bufs=1) as constp, \
             tc.tile_pool(name="xt", bufs=1) as xtp, \
             tc.tile_pool(name="rot", bufs=1) as rotp, \
             tc.tile_pool(name="wres", bufs=1) as wresp, \
             tc.tile_pool(name="ru", bufs=3) as rup, \
             tc.tile_pool(name="vext", bufs=1) as vxp, \
             tc.tile_pool(name="att", bufs=4) as attp, \
             tc.tile_pool(name="small", bufs=4) as smp, \
             tc.tile_pool(name="pairs", bufs=1) as pairp, \
             tc.tile_pool(name="atile", bufs=1) as atp, \
             tc.tile_pool(name="ysb", bufs=3) as ysbp, \
             tc.tile_pool(name="psSC", bufs=2, space="PSUM") as psSC, \
             tc.tile_pool(name="psPV", bufs=2, space="PSUM") as psPV, \
             tc.tile_pool(name="psA", bufs=2, space="PSUM") as psA:

            # ---- input DMAs: two queues in parallel. sync: batch-0 x +
            # k/v weights + consts (everything the prologue needs, ~1.3MB);
            # gpsimd: wq + batch-1 x + wo (6.8MB, needed later).
            xts = {}
            xstrip = xtp.tile([128, B * 8 * SEQ], bf16, tag="xt",
                              name="xstrip")
            for b in range(B):
                for kt in range(8):
                    off = (b * 8 + kt) * SEQ
                    xts[b, kt] = xstrip[:, off:off + SEQ]

            # Two DMA queues fed in parallel; tiny consts first on sync so
            # the bias-broadcast matmuls warm the PE immediately; batch-0 x
            # split across both queues in kt order so the k-projection can
            # start as soon as the first chunks + wk land.
            # gpsimd gets ONLY the early small transfers (its compute queue
            # must be free for rope muls by ~8us); sync carries everything
            # else. Early tensors split finely so the k-projection can start
            # as soon as its first operands land.
            bout_sb = constp.tile([1, E], f32r, tag="bouts", name="bout_sb")
            ones1 = constp.tile([1, 128], f32r, tag="ones1", name="ones1")
            nc.sync.dma_start(out=bout_sb, in_=bout_d[:, :])
            nc.sync.dma_start(out=ones1, in_=ones_d[:, :])
            wkr = wresp.tile([128, 2048], bf16, tag="wkr", name="wkr")
            nc.gpsimd.dma_start(out=wkr[:, 0:1024], in_=wk_d[:, 0:1024])
            nc.sync.dma_start(out=xstrip[:, 0:2 * SEQ],
                              in_=xt_d[:, 0:2 * SEQ])
            nc.gpsimd.dma_start(out=wkr[:, 1024:2048], in_=wk_d[:, 1024:2048])
            nc.sync.dma_start(out=xstrip[:, 2 * SEQ:4 * SEQ],
                              in_=xt_d[:, 2 * SEQ:4 * SEQ])
            nc.gpsimd.dma_start(out=xstrip[:, 4 * SEQ:6 * SEQ],
                                in_=xt_d[:, 4 * SEQ:6 * SEQ])
            nc.gpsimd.dma_start(out=xstrip[:, 6 * SEQ:8 * SEQ],
                                in_=xt_d[:, 6 * SEQ:8 * SEQ])
            wvr = wresp.tile([128, 1024], bf16, tag="wvr", name="wvr")
            nc.gpsimd.dma_start(out=wvr, in_=wv_d[:, :])

            tc_sb = constp.tile([128, SEQ], bf16, tag="tcs", name="tc_sb")
            ts_sb = constp.tile([128, SEQ], bf16, tag="tss", name="ts_sb")
            p32_sb = constp.tile([128, 128], bf16, tag="p32s", name="p32_sb")
            lam_sb = constp.tile([128, 2], f32, tag="lams", name="lam_sb")
            nc.gpsimd.dma_start(out=tc_sb, in_=tc_d[:, :])
            nc.gpsimd.dma_start(out=ts_sb, in_=ts_d[:, :])
            nc.gpsimd.dma_start(out=p32_sb, in_=p32_d[:, :])
            nc.gpsimd.dma_start(out=lam_sb, in_=lam_d[:, :])

            wqr = wresp.tile([128, 16384], bf16, tag="wqr", name="wqr")

            # chunk = mat*4 + mt//2; order so both mats of each mt-pair
            # arrive together, in mt order (all on sync).
            for chunk in (0, 4, 1, 5, 2, 6, 3, 7):
                nc.sync.dma_start(
                    out=wqr[:, chunk * 2048:(chunk + 1) * 2048],
                    in_=wq_d[chunk, :, :])
            nc.sync.dma_start(out=xstrip[:, 8 * SEQ:12 * SEQ],
                              in_=xt_d[:, 8 * SEQ:12 * SEQ])
            nc.sync.dma_start(out=xstrip[:, 12 * SEQ:16 * SEQ],
                              in_=xt_d[:, 12 * SEQ:16 * SEQ])
            wor = wresp.tile([128, 8192], bf16, tag="wor", name="wor")
            nc.sync.dma_start(out=wor, in_=wo_d[:, :])
            wo_sb = {}
            for kt in range(8):
                for nh in range(2):
                    off = kt * 1024 + nh * 512
                    wo_sb[kt, nh] = wor[:, off:off + 512]

            def wq_sb(mat, mt, kt):
                off = (mat * 4 + mt // 2) * 2048 + (mt % 2) * 1024 + kt * 128
                return wqr[:, off:off + 128]

            identf = constp.tile([128, 128], f32, tag="identf", name="identf")
            make_identity(nc, identf)
            identb = constp.tile([128, 128], bf16, tag="identb", name="identb")
            nc.vector.tensor_copy(identb, identf)

            # ---- shared state ----
            qrot, krot, vext, pairs, at2s = {}, {}, {}, {}, {}
            bias_sb = constp.tile([128, E], f32, tag="biasbc", name="bias_sb")

            # RoPE is split so its 4-engine chain never head-of-line blocks
            # the PE queue: the cast (ACT, frees the proj psum slot) is
            # emitted right after the projection matmuls; the rot part (GPS
            # muls -> PE P32 -> DVE add) is deferred until other PE work has
            # been queued.
            def rope_cast(psum_in, name):
                qps = rup.tile([128, SEQ], bf16, tag="ropec",
                               name=f"c_{name}")
                nc.scalar.activation(qps, psum_in, ACTF.Copy)
                return qps

            def rope_rot(qps, rot_out, name):
                t = rup.tile([128, SEQ], bf16, tag="ropet", name=f"t_{name}")
                u = rup.tile([128, SEQ], bf16, tag="ropeu", name=f"u_{name}")
                nc.gpsimd.tensor_mul(t, qps, tc_sb)
                nc.gpsimd.tensor_mul(u, qps, ts_sb)
                uswt = psA.tile([128, 512], f32, tag="A", name=f"usw_{name}")
                usw = uswt[:, 0:SEQ]
                nc.tensor.matmul(usw, p32_sb, u, start=True, stop=True)
                nc.vector.tensor_add(rot_out, t, usw)

            def qproj_cast(b, mt):
                casts = []
                for mat in range(2):
                    ps = psA.tile([128, 512], f32, tag="A",
                                  name=f"qp{b}_{mat}_{mt}")
                    qp = ps[:, 0:SEQ]
                    for kt in range(8):
                        nc.tensor.matmul(
                            qp, wq_sb(mat, mt, kt), xts[b, kt],
                            start=(kt == 0), stop=(kt == 7))
                    casts.append(rope_cast(qp, f"q{mat}_{b}_{mt}"))
                return casts

            def qproj_finish(b, mt, casts):
                for mat in range(2):
                    rot = rotp.tile([128, SEQ], bf16, tag=f"q{mat}_{b}_{mt}",
                                    name=f"qr{mat}_{b}_{mt}")
                    rope_rot(casts[mat], rot, f"q{mat}_{b}_{mt}")
                    qrot[mat, b, mt] = rot

            def kv_cast(b):
                st = {}
                for mat in range(2):
                    ps = psA.tile([128, 512], f32, tag="A",
                                  name=f"kp{b}_{mat}")
                    kp = ps[:, 0:SEQ]
                    for kt in range(8):
                        nc.tensor.matmul(
                            kp, wkr[:, mat * 1024 + kt * 128:
                                    mat * 1024 + kt * 128 + 128],
                            xts[b, kt],
                            start=(kt == 0), stop=(kt == 7))
                    st[f"k{mat}"] = rope_cast(kp, f"k{mat}_{b}")
                # v^T at full rate
                ps = psA.tile([128, 512], f32, tag="A", name=f"vp{b}")
                vtp = ps[:, 0:SEQ]
                for kt in range(8):
                    nc.tensor.matmul(
                        vtp, wvr[:, kt * 128:(kt + 1) * 128],
                        xts[b, kt],
                        start=(kt == 0), stop=(kt == 7))
                vt_sb = rup.tile([128, SEQ], bf16, tag="vtsb",
                                 name=f"vt_sb{b}")
                nc.vector.tensor_copy(vt_sb, vtp)
                st["vt"] = vt_sb
                return st

            def kv_finish(b, st):
                for mat in range(2):
                    rot = rotp.tile([128, SEQ], bf16, tag=f"k{mat}_{b}",
                                    name=f"kr{mat}_{b}")
                    rope_rot(st[f"k{mat}"], rot, f"k{mat}_{b}")
                    krot[mat, b] = rot
                vt_sb = st["vt"]
                for s in range(3):
                    vtr = psA.tile([128, 256], bf16, tag="A",
                                   name=f"vtr{b}_{s}")
                    nc.tensor.matmul(vtr[:, 0:128],
                                     vt_sb[:, s * 128:(s + 1) * 128],
                                     identb, is_transpose=True)
                    for g in range(2):
                        # vext layout: [1/lam | v(64) | 1]; branch0 rhs =
                        # cols 1:66 ([v|1]), branch1 rhs = cols 0:65
                        # ([1/lam|v]).
                        ve = vxp.tile([128, 66], bf16, tag=f"ve_{b}_{s}_{g}",
                                      name=f"ve{b}_{s}_{g}")
                        nc.vector.tensor_copy(ve[:, 1:65],
                                              vtr[:, g * 64:(g + 1) * 64])
                        nc.gpsimd.tensor_copy(ve[:, 0:1], lam_sb[:, 0:1])
                        nc.gpsimd.tensor_copy(ve[:, 65:66], lam_sb[:, 1:2])
                        vext[b, s, g] = ve

            def bias_broadcast():
                for nh in range(2):
                    bps = psA.tile([128, 512], f32, tag="A", name=f"bps{nh}")
                    nc.tensor.matmul(bps, ones1,
                                     bout_sb[:, nh * 512:(nh + 1) * 512],
                                     start=True, stop=True)
                    nc.vector.tensor_copy(
                        bias_sb[:, nh * 512:(nh + 1) * 512], bps)

            # ---- attention stages (software-pipelined per batch) ----

            def stage_A(b, i, hw, ctx):
                mt, h, w = hw
                g = h & 1
                base1 = 64 * g
                base2 = 64 - base1
                # both branches' scores in one 2-bank psum tile so a single
                # N=1024 exp covers the whole head-window.
                st_ps = psSC.tile([128, 1024], f32, tag="sc",
                                  name=f"sc{b}_{i}")
                for br, qb in ((0, base1), (1, base2)):
                    mms = []
                    for kts in range(2):
                        mms.append(nc.tensor.matmul(
                            st_ps[:, br * 512 + kts * 256:
                                  br * 512 + kts * 256 + 256],
                            krot[br, b][qb:qb + 64,
                                        w * 128 + kts * 128:
                                        w * 128 + kts * 128 + 128],
                            qrot[br, b, mt][qb:qb + 64,
                                            w * 128:w * 128 + 256],
                            start=(kts == 0), stop=(kts == 1)))
                    order_group(mms)
                ctx[i] = {"sc": st_ps}

            def stage_B(b, i, hw, ctx):
                e = attp.tile([128, 1024], bf16, tag="e",
                              name=f"e{b}_{i}")
                nc.scalar.activation(e, ctx[i]["sc"], ACTF.Exp,
                                     scale=0.125)
                ctx[i]["e"] = e

            def stage_CD(b, i, hw, ctx):
                mt, h, w = hw
                g = h & 1
                e = ctx[i]["e"]
                pv = psPV.tile([128, 260], f32, tag="pv", name=f"pv{b}_{i}")
                mms = []
                first = True
                for kts in range(2):
                    for qt in range(2):
                        for br in range(2):
                            col = qt * 130 + br * 65
                            ve = vext[b, w + kts, g]
                            rhs = ve[:, 0:65] if br == 1 else ve[:, 1:66]
                            mms.append(nc.tensor.matmul(
                                pv[:, col:col + 65],
                                e[:, br * 512 + kts * 256 + qt * 128:
                                  br * 512 + kts * 256 + qt * 128 + 128],
                                rhs,
                                start=first,
                                stop=(kts == 1 and qt == 1 and br == 1)))
                            first = False
                order_group(mms)

                # denominators: cols 64,65 (qt0: r1, lam*r2) and 194,195
                r_sb = smp.tile([128, 4], f32, tag="recip", name=f"r{b}_{i}")
                nc.vector.reciprocal(
                    r_sb.rearrange("p (c k) -> p c k", c=2),
                    pv.rearrange("p (c k) -> p c k", c=2)[:, :, 64:66])
                if g == 0:
                    for qt in range(2):
                        pairs[b, mt, w, qt] = pairp.tile(
                            [128, 128], bf16, tag=f"pair{b}_{mt}_{w}_{qt}",
                            name=f"pair{b}_{mt}_{w}_{qt}")
                for qt in range(2):
                    t1 = smp.tile([128, 64], bf16, tag=f"t1_{qt}",
                                  name=f"t1_{b}_{i}_{qt}")
                    nc.vector.tensor_scalar_mul(
                        t1, pv[:, qt * 130:qt * 130 + 64],
                        r_sb[:, 2 * qt:2 * qt + 1])
                    # pair = (pv_br2 * lam*r2) - pv_br1*r1 = -(a); Wout is
                    # negated on the host to compensate.
                    nc.vector.scalar_tensor_tensor(
                        out=pairs[b, mt, w, qt][:, g * 64:(g + 1) * 64],
                        in0=pv[:, qt * 130 + 66:qt * 130 + 130],
                        scalar=r_sb[:, 2 * qt + 1:2 * qt + 2],
                        in1=t1, op0=ALU.mult, op1=ALU.subtract)

            def stage_E(b, mt, w):
                tr = psA.tile([128, 256], bf16, tag="A", name=f"tr{b}_{mt}_{w}")
                for qt in range(2):
                    nc.tensor.transpose(tr[:, qt * 128:(qt + 1) * 128],
                                        pairs[b, mt, w, qt], identb)
                at = atp.tile([128, 256], bf16, tag=f"at{b}_{w}_{mt}",
                              name=f"at{b}_{w}_{mt}")
                nc.vector.tensor_copy(at, tr)
                at2s[b, w, mt] = at

            def run_pipeline(b, fillers=None, post_e=None):
                hws = [(mt, 2 * mt + gg, w)
                       for mt in range(8) for gg in (0, 1) for w in (0, 1)]
                n = len(hws)
                ctx = {}
                pend = {}
                for i in range(n + 2):
                    if i < n:
                        stage_A(b, i, hws[i], ctx)
                    if 0 <= i - 1 < n:
                        stage_B(b, i - 1, hws[i - 1], ctx)
                    if i < n:
                        # q-projection prefetch, two groups ahead; emitted
                        # after this step's exps so the ACT cast doesn't
                        # delay them.
                        mt, h, w = hws[i]
                        if h == 2 * mt and w == 0:
                            if mt + 1 in pend:
                                qproj_finish(b, mt + 1, pend.pop(mt + 1))
                            if mt + 2 < 8:
                                pend[mt + 2] = qproj_cast(b, mt + 2)
                            if fillers and mt in fillers:
                                for thunk in fillers[mt]:
                                    thunk()
                    if 0 <= i - 2 < n:
                        j = i - 2
                        stage_CD(b, j, hws[j], ctx)
                        mt, h, w = hws[j]
                        if h % 2 == 1:
                            stage_E(b, mt, w)
                            if post_e and (mt, w) in post_e:
                                post_e[mt, w]()
                        del ctx[j]

            def outproj(b, w):
                for qt in range(2):
                    for nh in range(2):
                        y_ps = psA.tile([128, 512], f32, tag="A",
                                        name=f"y{b}_{w}_{qt}_{nh}")
                        mms = []
                        for kt in range(8):
                            mms.append(nc.tensor.matmul(
                                y_ps,
                                at2s[b, w, kt][:, qt * 128:(qt + 1) * 128],
                                wo_sb[kt, nh],
                                start=(kt == 0), stop=(kt == 7)))
                        order_group(mms)
                        y_sb = ysbp.tile([128, 512], bf16, tag="ysb",
                                         name=f"ysb{b}_{w}_{qt}_{nh}")
                        nc.vector.tensor_add(
                            y_sb, y_ps,
                            bias_sb[:, nh * 512:(nh + 1) * 512])
                        nc.sync.dma_start(
                            out=y_d[b, (w * 2 + qt) * 128:
                                    (w * 2 + qt) * 128 + 128,
                                    nh * 512:(nh + 1) * 512],
                            in_=y_sb)

            # ---- emission schedule ----
            bias_broadcast()
            st0 = kv_cast(0)
            c00 = qproj_cast(0, 0)
            c01 = qproj_cast(0, 1)
            kv_finish(0, st0)
            qproj_finish(0, 0, c00)
            qproj_finish(0, 1, c01)
            # batch-1 projections interleave with batch-0's last pipeline
            # groups so the batch boundary has no ACT/DVE idle stretch.
            bx = {}
            fillers = {
                5: [lambda: bx.__setitem__("st1", kv_cast(1))],
                6: [lambda: bx.__setitem__("c10", qproj_cast(1, 0)),
                    lambda: kv_finish(1, bx["st1"])],
                7: [lambda: bx.__setitem__("c11", qproj_cast(1, 1)),
                    lambda: qproj_finish(1, 0, bx["c10"])],
            }
            # batch-0 w0's out-projection starts inside pipeline(0) as soon
            # as its last at-tile is ready; w1's interleaves into batch-1's
            # first pipeline group.
            run_pipeline(0, fillers,
                         post_e={(7, 0): lambda: outproj(0, 0)})
            qproj_finish(1, 1, bx["c11"])
            run_pipeline(1,
                         fillers={0: [lambda: outproj(0, 1)]},
                         post_e={(7, 0): lambda: outproj(1, 0)})
            outproj(1, 1)

    split_matmul_waits()
    return nc


def get_program():
    if "nc" not in _PROGRAM_CACHE:
        _PROGRAM_CACHE["nc"] = _build_program()
    return _PROGRAM_CACHE["nc"]


# ------------------------------------------------------------------ host API

def make_in_maps(x, Wq1, Wq2, Wk1, Wk2, Wv, Wout, bout, lq1, lk1, lq2, lk2):
    import ml_dtypes
    bf16 = ml_dtypes.bfloat16

    x = np.asarray(x, dtype=np.float32)
    lam = float(np.clip(
        np.exp(np.asarray(lq1, np.float64) @ np.asarray(lk1, np.float64))
        - np.exp(np.asarray(lq2, np.float64) @ np.asarray(lk2, np.float64))
        + LAMBDA_INIT, 0.1, 0.9))

    qp1, qp2 = _head_perm(), _q2_perm()
    kp1, kp2 = _k_perm(False), _k_perm(True)

    wq_t = np.stack([
        _tile_w(np.asarray(Wq1, np.float32)[:, qp1], 8, 8),
        _tile_w(np.asarray(Wq2, np.float32)[:, qp2], 8, 8),
    ])  # (mat, kt, mt, 128, 128)
    # chunk layout for single contiguous DMAs:
    # (mat, mt//2, p, mt%2, kt, s) -> (8, 128, 2048)
    wq = np.ascontiguousarray(
        wq_t.reshape(2, 8, 4, 2, 128, 128)
        .transpose(0, 2, 4, 3, 1, 5).reshape(8, 128, 2048)).astype(bf16)
    # wk/wv/wo partition-major contiguous: [128, ...]
    wk = np.ascontiguousarray(np.stack([
        _tile_w(np.asarray(Wk1, np.float32)[:, kp1], 8, 1)[:, 0],
        _tile_w(np.asarray(Wk2, np.float32)[:, kp2], 8, 1)[:, 0],
    ]).transpose(2, 0, 1, 3).reshape(128, 2048)).astype(bf16)
    wv = np.ascontiguousarray(
        _tile_w(np.asarray(Wv, np.float32), 8, 1)[:, 0]
        .transpose(1, 0, 2).reshape(128, 1024)).astype(bf16)
    # NOTE: negated — the device computes -a (branch2-scaled minus branch1).
    wo = np.ascontiguousarray(
        _tile_w(-np.asarray(Wout, np.float32), 8, 2)
        .transpose(2, 0, 1, 3).reshape(128, 8192)).astype(bf16)
    boutv = np.asarray(bout, np.float32).reshape(1, E)

    lamv = np.zeros((128, 2), np.float32)
    lamv[:, 0] = 1.0 / lam  # branch-2 "ones" column -> den2/lam
    lamv[:, 1] = 1.0        # branch-1 ones column

    # x^T, tiled: (B, 8, 128, SEQ) per core
    xT = np.ascontiguousarray(x.transpose(0, 2, 1))  # (B, E, L)

    in_maps = []
    for c in range(NCORES):
        s0 = 256 * c
        xt = np.ascontiguousarray(
            xT[:, :, s0:s0 + SEQ].reshape(B, 8, 128, SEQ)
            .transpose(2, 0, 1, 3).reshape(128, B * 8 * SEQ)).astype(bf16)
        tct, tst = _trig_tables(c)
        in_maps.append({
            "xt": xt, "wq": wq, "wk": wk, "wv": wv, "wo": wo,
            "tct": tct.astype(bf16), "tst": tst.astype(bf16),
            "lamv": lamv, "boutv": boutv,
            "p32": _p32().astype(bf16),
            "onesv": np.ones((1, 128), np.float32),
        })
    return in_maps


def kernel(**inputs) -> np.ndarray:
    from concourse.bass_utils import run_bass_kernel_spmd

    in_maps = make_in_maps(**inputs)
    nc = get_program()
    res = run_bass_kernel_spmd(nc, in_maps, core_ids=list(range(NCORES)))
    out = np.empty((B, L, E), dtype=np.float32)
    for c in range(NCORES):
        out[:, 512 * c:512 * (c + 1), :] = \
            res.results[c]["y"].astype(np.float32)
    return out
